# revision 1
# baseline (speedup 1.0000x reference)
"""Trainium2 Bass kernel for nn_Attention_org_cross (cross-modal channel attention).

Sharding: 8 cores = 4 batches x 2 modality directions (pure data parallel).
Core (b, side=0): optical queries attend to DSM K/V -> out[b,:,0:960]; side=1 reverse.

bf16 data path (PSUM f32), p-major DRAM layouts, wq folded into the host-side
eq pack (eq' = Q/sqrt(KV)), so phase 1 computes Ut = kv^T-as-lhsT @ eq' directly:
  1: stream kv/eq'; Ut[h] (kv-c x d) in PSUM; PE-transpose kv->kvT (paired evac)
  2: s = Ut^T wkt (col 240 = free row-sums); block var -> r; exp(r*s) (shift-free
     softmax, inorm makes it safe); pt; P2t  -- step-interleaved across heads
  3+4a fused per f-block: ctxT -> t4[h] (s4, 1/esum-scaled) + tc1[h] ([s3|s2|s1]);
     out s4-half = sum_h t4 @ w4 -> store cols 448:960
  then: DMA-repack tc1 into b01/b23/cpk/dpk; out small-half -> store cols 0:448
"""
import sys

sys.path.insert(0, "/opt/trn_rl_repo")

import numpy as np
import ml_dtypes

import concourse.bacc as bacc
import concourse.mybir as mybir
import concourse.tile as tile
from concourse.bass_utils import run_bass_kernel_spmd

F32 = mybir.dt.float32
BF16 = mybir.dt.bfloat16
BF = ml_dtypes.bfloat16
AX = mybir.AxisListType.X
MULT = mybir.AluOpType.mult
SUB = mybir.AluOpType.subtract
ADD = mybir.AluOpType.add
AExp = mybir.ActivationFunctionType.Exp
ASqrt = mybir.ActivationFunctionType.Sqrt

B, N, H, KV = 4, 4096, 4, 960
CQ = (16, 32, 64, 128)
RAW = (0, 64, 192, 448)
QOFF = (224, 192, 128, 0)        # scale i -> within-head q offset ([s4 s3 s2 s1])
KOFF = (0, 16, 48, 112)          # kv-side within-head offsets ([s1 s2 s3 s4])
EPS = 1e-5
NT = 32
NGRP = 2
NF = 8

_CACHE = {}


def _build_bass():
    nc = bacc.Bacc(trn_type="TRN2", target_bir_lowering=False, debug=False)

    eqp = nc.declare_dram_parameter("eqp", [128, NT * 960], BF16, isOutput=False)
    kvp = nc.declare_dram_parameter("kvp", [128, NT * 960], BF16, isOutput=False)
    # wall: idt 128 | wkt 482 | wvs 480 | w4 2048 | w3 512 | w2 128 | w1 64 | zz 512
    wall = nc.declare_dram_parameter("wall", [128, 4546], BF16, isOutput=False)
    indb = nc.declare_dram_parameter("indb", [128, 8], F32, isOutput=False)
    indc = nc.declare_dram_parameter("indc", [4, 240], F32, isOutput=False)
    outp = nc.declare_dram_parameter("outp", [128, NT * 960], BF16, isOutput=True)

    eq_r = eqp.rearrange("p (t c) -> p t c", t=NT)
    kv_r = kvp.rearrange("p (t c) -> p t c", t=NT)
    out_r = outp.rearrange("p (t c) -> p t c", t=NT)

    def cp(e, out, in_):
        e %= 3
        if e == 0:
            nc.vector.tensor_copy(out, in_)
        elif e == 1:
            nc.scalar.copy(out, in_)
        else:
            nc.gpsimd.tensor_copy(out, in_)

    def cpmul(e, out, in_, s):
        e %= 3
        if e == 0:
            nc.vector.tensor_scalar(out=out, in0=in_, scalar1=s, scalar2=None,
                                    op0=MULT)
        elif e == 1:
            nc.scalar.mul(out, in_, s)
        else:
            nc.gpsimd.tensor_scalar(out=out, in0=in_, scalar1=s, scalar2=None,
                                    op0=MULT)

    with tile.TileContext(nc) as tc:
        from contextlib import ExitStack
        with ExitStack() as outer:
            wts = outer.enter_context(tc.tile_pool(name="wts", bufs=1))
            kvtp = outer.enter_context(tc.tile_pool(name="kvtp", bufs=1))
            p2sb = outer.enter_context(tc.tile_pool(name="p2sb", bufs=1))
            gsbp = outer.enter_context(tc.tile_pool(name="gsbp", bufs=1))
            esp = outer.enter_context(tc.tile_pool(name="esp", bufs=1))
            gstack = ExitStack()
            gbp = gstack.enter_context(tc.tile_pool(name="gbp", bufs=1, space="PSUM"))

            wall_sb = wts.tile([128, 4546], BF16, tag="wall")
            nc.sync.dma_start(out=wall_sb[:, 0:128], in_=wall[:, 0:128])
            nc.vector.memset(wall_sb[0:1, 4034:4546], 0.0)
            idt = wall_sb[:, 0:128]
            wkt_sb = wall_sb[:, 128:610].rearrange("p (u c) -> p u c", u=2)
            wvs_sb = wall_sb[:, 610:1090].rearrange("p (u c) -> p u c", u=2)
            w4_sb = wall_sb[:, 1090:3138].rearrange("p (u c) -> p u c", u=4)
            w3_sb = wall_sb[:, 3138:3650].rearrange("p (u c) -> p u c", u=2)
            wcdA_sb = wall_sb[:, 3650:3842]
            wcdB_sb = wall_sb[:, 3842:4034]
            zzt = wall_sb[0:1, 4034:4546]
            indb_sb = wts.tile([128, 8], F32, tag="indb")
            indc_sb = wts.tile([4, 240], F32, tag="indc")
            eps_t = wts.tile([4, 1], F32, tag="eps")
            nc.scalar.dma_start(out=indb_sb, in_=indb[:, :])
            nc.scalar.dma_start(out=indc_sb, in_=indc[:, :])
            nc.vector.memset(eps_t, EPS)
            # preload the sqrt act table off the critical path
            sqd = wts.tile([4, 1], F32, tag="sqd")
            nc.scalar.activation(out=sqd, in_=eps_t, func=ASqrt, bias=eps_t)

            kvT = [kvtp.tile([128, 2, N], BF16, tag=f"kvt{h}", name=f"kvt{h}")
                   for h in range(H)]
            P2t = [p2sb.tile([128, 2, 240], BF16, tag=f"p2t{h}", name=f"p2t{h}")
                   for h in range(H)]
            gsb = [gsbp.tile([128, 2, 240], BF16, tag=f"gsb{h}", name=f"gsb{h}")
                   for h in range(H)]
            esr = [esp.tile([128, 2], F32, tag=f"esr{h}", name=f"esr{h}")
                   for h in range(H)]
            gb = [gbp.tile([128, 480], F32, tag=f"g{h}", name=f"g{h}")
                  for h in range(H)]

            # ---- phase 1: Ut accumulation (lhsT=kv) + kv transposes ----
            with tc.tile_pool(name="stream", bufs=3) as stream, \
                 tc.tile_pool(name="tps", bufs=4, space="PSUM") as tps:
                for h in range(H):
                    nc.tensor.matmul(gb[h], zzt[0:1, 0:128], zzt[0:1, 0:480],
                                     start=True, stop=False)
                def transposes(g, kv_t):
                    j0 = g * NGRP
                    for h in range(H):
                        hk = h * 240
                        tp = tps.tile([128, 2, NGRP, 128], BF16, tag="tp",
                                      name=f"tp{g}_{h}")
                        for jj in range(NGRP):
                            nc.tensor.transpose(tp[:, 0, jj, :],
                                                kv_t[:, jj, hk:hk + 128], idt)
                            nc.tensor.transpose(tp[0:112, 1, jj, :],
                                                kv_t[:, jj, hk + 128:hk + 240], idt)
                        e = (0, 0, 1, 1)[h]
                        cs = slice(j0 * 128, (j0 + NGRP) * 128)
                        cp(e, kvT[h][:, 0, cs],
                           tp.rearrange("p u j c -> p u (j c)")[:, 0, :])
                        cp(e, kvT[h][0:112, 1, cs],
                           tp.rearrange("p u j c -> p u (j c)")[0:112, 1, :])

                deferred = []
                for g in range(NT // NGRP):
                    eq_t = stream.tile([128, NGRP, 960], BF16, tag="eq")
                    kv_t = stream.tile([128, NGRP, 960], BF16, tag="kv")
                    j0 = g * NGRP
                    nc.sync.dma_start(out=kv_t, in_=kv_r[:, j0:j0 + NGRP, :])
                    nc.sync.dma_start(out=eq_t, in_=eq_r[:, j0:j0 + NGRP, :])
                    if g == 14:
                        nc.scalar.dma_start(out=wall_sb[:, 128:1090],
                                            in_=wall[:, 128:1090])
                    for h in range(H):
                        hq = hk = h * 240
                        for jj in range(NGRP):
                            nc.tensor.matmul(
                                gb[h][:, 0:240],
                                kv_t[:, jj, hk:hk + 128],
                                eq_t[:, jj, hq:hq + 240],
                                start=False, stop=False)
                            nc.tensor.matmul(
                                gb[h][0:112, 240:480],
                                kv_t[:, jj, hk + 128:hk + 240],
                                eq_t[:, jj, hq:hq + 240],
                                start=False, stop=False)
                    transposes(g, kv_t)

                for h in range(H):
                    nc.tensor.matmul(gb[h], zzt[0:1, 0:128], zzt[0:1, 0:480],
                                     start=False, stop=True)
                for g, kv_t in deferred:
                    transposes(g, kv_t)

            # big output-projection weights: transfer during the DMA-idle window
            nc.scalar.dma_start(out=wall_sb[:, 1090:4034], in_=wall[:, 1090:4034])

            # ---- Ut evac, free banks ----
            for h in range(H):
                cp(h % 2, gsb[h][:, 0, :], gb[h][:, 0:240])
                cp((h + 1) % 2, gsb[h][0:112, 1, :], gb[h][0:112, 240:480])
            gstack.close()

            # ---- phase 2: step-interleaved ----
            with tc.tile_pool(name="phw", bufs=5, space="PSUM") as phw, \
                 tc.tile_pool(name="phb", bufs=3, space="PSUM") as phb, \
                 tc.tile_pool(name="sm", bufs=1) as sm:
                scp, st0, st1, tiny, blk, var, rall = {}, {}, {}, {}, {}, {}, {}
                rv, pr0, pr1, ptp, pt, sq0, sq1 = {}, {}, {}, {}, {}, {}, {}
                for h in range(H):
                    scp[h] = phw.tile([128, 2, 246], F32, tag="w", name=f"scp{h}")
                    tiny[h] = scp[h][:, 0, 242:246]
                    nc.tensor.matmul(scp[h][:, 0, 0:241], gsb[h][:, 0, 0:128],
                                     wkt_sb[:, 0, :], start=True, stop=False)
                    nc.tensor.matmul(scp[h][:, 0, 0:241], gsb[h][0:112, 1, 0:128],
                                     wkt_sb[0:112, 1, :], start=False, stop=True)
                    nc.tensor.matmul(scp[h][0:112, 1, 0:241], gsb[h][:, 0, 128:240],
                                     wkt_sb[:, 0, :], start=True, stop=False)
                    nc.tensor.matmul(scp[h][0:112, 1, 0:241],
                                     gsb[h][0:112, 1, 128:240],
                                     wkt_sb[0:112, 1, :], start=False, stop=True)
                for h in range(H):
                    st0[h] = sm.tile([128, 2], F32, tag=f"st0{h}", name=f"st0{h}")
                    st1[h] = sm.tile([112, 2], F32, tag=f"st1{h}", name=f"st1{h}")
                    nc.vector.tensor_copy(st0[h][:, 0:1], scp[h][:, 0, 240:241])
                    nc.vector.tensor_copy(st1[h][:, 0:1], scp[h][0:112, 1, 240:241])
                    sq0[h] = sm.tile([128, 240], BF16, tag=f"sq0{h}", name=f"sq0{h}")
                    sq1[h] = sm.tile([112, 240], BF16, tag=f"sq1{h}", name=f"sq1{h}")
                    nc.scalar.activation(out=sq0[h], in_=scp[h][:, 0, 0:240],
                                         func=mybir.ActivationFunctionType.Square,
                                         accum_out=st0[h][:, 1:2])
                    nc.scalar.activation(out=sq1[h], in_=scp[h][0:112, 1, 0:240],
                                         func=mybir.ActivationFunctionType.Square,
                                         accum_out=st1[h][:, 1:2])
                for h in range(H):
                    nc.tensor.matmul(tiny[h][0:4, 2:4], indb_sb[:, 0:4], st0[h],
                                     start=True, stop=False)
                    nc.tensor.matmul(tiny[h][0:4, 2:4], indb_sb[0:112, 4:8], st1[h],
                                     start=False, stop=True)
                for h in range(H):
                    # indb is pre-scaled by 1/(nblk*240): tiny holds (mean, meansq)
                    var[h] = sm.tile([4, 1], F32, tag=f"var{h}", name=f"var{h}")
                    nc.scalar.activation(out=var[h], in_=tiny[h][0:4, 2:3],
                                         func=mybir.ActivationFunctionType.Square)
                    nc.vector.tensor_tensor(out=var[h], in0=tiny[h][0:4, 3:4],
                                            in1=var[h], op=SUB)
                for h in range(H):
                    rall[h] = sm.tile([4, 1], F32, tag=f"rall{h}", name=f"rall{h}")
                    nc.scalar.activation(out=rall[h], in_=var[h], func=ASqrt,
                                         bias=eps_t)
                    nc.vector.reciprocal(out=rall[h], in_=rall[h])
                for h in range(H):
                    nc.tensor.matmul(tiny[h][:, 0:1], indc_sb[:, 0:128], rall[h],
                                     start=True, stop=True)
                    nc.tensor.matmul(tiny[h][0:112, 1:2], indc_sb[:, 128:240],
                                     rall[h], start=True, stop=True)
                    rv[h] = sm.tile([128, 2], F32, tag=f"rv{h}", name=f"rv{h}")
                    nc.vector.tensor_copy(rv[h][:, 0:1], tiny[h][:, 0:1])
                    nc.vector.tensor_copy(rv[h][0:112, 1:2], tiny[h][0:112, 1:2])
                for h in range(H):
                    # inorm guarantees ~unit-variance scores: shift-free softmax
                    pr0[h] = sm.tile([128, 240], BF16, tag=f"pr0{h}", name=f"pr0{h}")
                    pr1[h] = sm.tile([112, 240], BF16, tag=f"pr1{h}", name=f"pr1{h}")
                    nc.scalar.activation(out=pr0[h], in_=scp[h][:, 0, 0:240],
                                         func=AExp, scale=rv[h][:, 0:1],
                                         accum_out=esr[h][:, 0:1])
                    nc.scalar.activation(out=pr1[h], in_=scp[h][0:112, 1, 0:240],
                                         func=AExp, scale=rv[h][0:112, 1:2],
                                         accum_out=esr[h][0:112, 1:2])
                    nc.vector.reciprocal(out=esr[h][:, 0:1], in_=esr[h][:, 0:1])
                    nc.vector.reciprocal(out=esr[h][0:112, 1:2],
                                         in_=esr[h][0:112, 1:2])
                for h in range(H):
                    ptp[h] = phb.tile([128, 2, 240], BF16, tag="b", name=f"ptp{h}")
                    nc.tensor.transpose(ptp[h][:, 0, 0:128], pr0[h][:, 0:128], idt)
                    nc.tensor.transpose(ptp[h][0:112, 1, 0:128], pr0[h][:, 128:240],
                                        idt)
                    nc.tensor.transpose(ptp[h][:, 0, 128:240], pr1[h][:, 0:128],
                                        idt[0:112, 0:112])
                    nc.tensor.transpose(ptp[h][0:112, 1, 128:240],
                                        pr1[h][:, 128:240], idt[0:112, 0:112])
                    pt[h] = sm.tile([128, 2, 240], BF16, tag=f"pt{h}", name=f"pt{h}")
                    cp(0, pt[h][:, 0, :], ptp[h][:, 0, :])
                    cp(1, pt[h][0:112, 1, :], ptp[h][0:112, 1, :])
                for h in range(H):
                    p2p = phw.tile([128, 2, 246], F32, tag="w", name=f"p2p{h}")
                    nc.tensor.matmul(p2p[:, 0, 0:240], wvs_sb[:, 0, 0:128],
                                     pt[h][:, 0, :], start=True, stop=False)
                    nc.tensor.matmul(p2p[:, 0, 0:240], wvs_sb[0:112, 1, 0:128],
                                     pt[h][0:112, 1, :], start=False, stop=True)
                    nc.tensor.matmul(p2p[0:112, 1, 0:240], wvs_sb[:, 0, 128:240],
                                     pt[h][:, 0, :], start=True, stop=False)
                    nc.tensor.matmul(p2p[0:112, 1, 0:240], wvs_sb[0:112, 1, 128:240],
                                     pt[h][0:112, 1, :], start=False, stop=True)
                    cp(0, P2t[h][:, 0, :], p2p[:, 0, 0:240])
                    cp(1, P2t[h][0:112, 1, :], p2p[0:112, 1, 0:240])

            # ---- phases 3+4 fully fused per f-block ----
            ctg = outer.enter_context(tc.tile_pool(name="ctg", bufs=1))
            ostp = outer.enter_context(tc.tile_pool(name="ostp", bufs=3))
            t4 = [ctg.tile([128, N], BF16, tag=f"t4{h}", name=f"t4{h}")
                  for h in range(H)]
            tc1 = [ctg.tile([112, N], BF16, tag=f"tc1{h}", name=f"tc1{h}")
                   for h in range(H)]
            b01 = ctg.tile([128, N], BF16, tag="b01")
            b23 = ctg.tile([128, N], BF16, tag="b23")
            cdA = ctg.tile([128, N], BF16, tag="cdA")
            cdB = ctg.tile([128, N], BF16, tag="cdB")
            nc.vector.memset(cdA, 0.0)
            nc.gpsimd.memset(cdB, 0.0)
            with tc.tile_pool(name="cps", bufs=3, space="PSUM") as cps, \
                 tc.tile_pool(name="cps1", bufs=2, space="PSUM") as cps1, \
                 tc.tile_pool(name="ops", bufs=3, space="PSUM") as ops:
                def ph3_block(f):
                    fc = slice(f * 512, (f + 1) * 512)
                    for h in range(H):
                        c0 = cps.tile([128, 512], F32, tag="c0")
                        c1 = cps1.tile([112, 512], F32, tag="c1")
                        nc.tensor.matmul(c0, P2t[h][:, 0, 0:128],
                                         kvT[h][:, 0, fc], start=True, stop=False)
                        nc.tensor.matmul(c0, P2t[h][0:112, 1, 0:128],
                                         kvT[h][0:112, 1, fc], start=False, stop=True)
                        nc.tensor.matmul(c1, P2t[h][:, 0, 128:240],
                                         kvT[h][:, 0, fc], start=True, stop=False)
                        nc.tensor.matmul(c1, P2t[h][0:112, 1, 128:240],
                                         kvT[h][0:112, 1, fc], start=False, stop=True)
                        o = (h % 2) * 64
                        bt = b01 if h < 2 else b23
                        cd = cdA if h < 2 else cdB
                        cpmul((h + f + 1) % 2, t4[h][:, fc], c0, esr[h][:, 0:1])
                        cpmul((h + f) % 2, tc1[h][:, fc], c1,
                              esr[h][0:112, 1:2])
                        cp(2, bt[o:o + 64, fc], tc1[h][0:64, fc])
                        cp(2, cd[o:o + 48, fc], tc1[h][64:112, fc])

                def ph4_block(f, interleave=False):
                    ost = ostp.tile([128, 4, 960], BF16, tag="ost")
                    if interleave:
                        for q in range(4):
                            j = 4 * f + q
                            ncol = slice(j * 128, (j + 1) * 128)
                            bp = ops.tile([128, 512], F32, tag="o", name=f"bp{j}")
                            for hh in range(H):
                                nc.tensor.matmul(bp, t4[hh][:, ncol],
                                                 w4_sb[:, hh, :],
                                                 start=(hh == 0), stop=(hh == 3))
                            cp(q % 2, ost[:, q, 448:960], bp)
                            ap = ops.tile([128, 512], F32, tag="o", name=f"ap{j}")
                            nc.tensor.matmul(ap[:, 192:448], b01[:, ncol],
                                             w3_sb[:, 0, :], start=True, stop=False)
                            nc.tensor.matmul(ap[:, 192:448], b23[:, ncol],
                                             w3_sb[:, 1, :], start=False, stop=True)
                            nc.tensor.matmul(ap[:, 0:192], cdA[:, ncol], wcdA_sb,
                                             start=True, stop=False)
                            nc.tensor.matmul(ap[:, 0:192], cdB[:, ncol], wcdB_sb,
                                             start=False, stop=True)
                            cp((q + 1) % 2, ost[:, q, 0:448], ap[:, 0:448])
                            nc.sync.dma_start(out=out_r[:, j:j + 1, :],
                                              in_=ost[:, q:q + 1, :])
                        return
                    for q in range(4):
                        j = 4 * f + q
                        ncol = slice(j * 128, (j + 1) * 128)
                        bp = ops.tile([128, 512], F32, tag="o", name=f"bp{j}")
                        for hh in range(H):
                            nc.tensor.matmul(bp, t4[hh][:, ncol], w4_sb[:, hh, :],
                                             start=(hh == 0), stop=(hh == 3))
                        cp(q % 2, ost[:, q, 448:960], bp)
                    for q in range(4):
                        j = 4 * f + q
                        ncol = slice(j * 128, (j + 1) * 128)
                        ap = ops.tile([128, 512], F32, tag="o", name=f"ap{j}")
                        nc.tensor.matmul(ap[:, 192:448], b01[:, ncol], w3_sb[:, 0, :],
                                         start=True, stop=False)
                        nc.tensor.matmul(ap[:, 192:448], b23[:, ncol], w3_sb[:, 1, :],
                                         start=False, stop=True)
                        nc.tensor.matmul(ap[:, 0:192], cdA[:, ncol], wcdA_sb,
                                         start=True, stop=False)
                        nc.tensor.matmul(ap[:, 0:192], cdB[:, ncol], wcdB_sb,
                                         start=False, stop=True)
                        cp((q + 1) % 2, ost[:, q, 0:448], ap[:, 0:448])
                        if f == NF - 1:
                            nc.sync.dma_start(out=out_r[:, j:j + 1, :],
                                              in_=ost[:, q:q + 1, :])
                        elif q % 2 == 1:
                            nc.sync.dma_start(
                                out=out_r[:, j - 1:j + 1, :],
                                in_=ost[:, q - 1:q + 1, :])

                # software-pipelined by one f-block: ph4 consumes f-1 while
                # ph3 produces f
                for step in range(NF + 1):
                    if step < NF:
                        ph3_block(step)
                    if step >= 1:
                        ph4_block(step - 1, interleave=(step == NF))
    nc.finalize()
    return nc


def _host_pack(inputs, b, side):
    if side == 0:
        embs = [inputs['emb1'], inputs['emb2'], inputs['emb3'], inputs['emb4']]
        wq = [inputs[f'wq{i+1}'] for i in range(4)]
        kvsrc = inputs['emb_alld']
    else:
        embs = [inputs['embd1'], inputs['embd2'], inputs['embd3'], inputs['embd4']]
        wq = [inputs[f'wqd{i+1}'] for i in range(4)]
        kvsrc = inputs['emb_all']
    scale = np.float32(1.0 / np.sqrt(np.float32(KV)))
    eq = np.empty((N, 960), np.float32)
    kvf = np.empty((N, 960), np.float32)
    for h in range(H):
        for i in range(4):
            cq = CQ[i]
            blkq = np.asarray(embs[i][b][:, h * cq:(h + 1) * cq], np.float32)
            # fold wq (and 1/sqrt(KV)) into the q-side pack: Q = emb @ wq^T
            eq[:, h * 240 + QOFF[i]: h * 240 + QOFF[i] + cq] = \
                (blkq @ np.asarray(wq[i][h], np.float32).T) * scale
            kvf[:, h * 240 + KOFF[i]: h * 240 + KOFF[i] + cq] = \
                kvsrc[b][:, RAW[i] + h * cq: RAW[i] + (h + 1) * cq]
    eqp = np.ascontiguousarray(
        eq.reshape(NT, 128, 960).transpose(1, 0, 2)).reshape(128, NT * 960)
    kvp = np.ascontiguousarray(
        kvf.reshape(NT, 128, 960).transpose(1, 0, 2)).reshape(128, NT * 960)
    return eqp.astype(BF), kvp.astype(BF)


def _host_weights(inputs, side):
    if side == 0:
        wk, wv = inputs['wkd'], inputs['wvd']
        wout = [inputs[f'wout{i+1}'] for i in range(4)]
    else:
        wk, wv = inputs['wk'], inputs['wv']
        wout = [inputs[f'woutd{i+1}'] for i in range(4)]
    wkT = np.asarray(wk).T.astype(np.float32)
    wkt = np.zeros((128, 2, 241), np.float32)
    wkt[:, 0, 0:240] = wkT[0:128, :]
    wkt[0:112, 1, 0:240] = wkT[128:240, :]
    wkt[:, 0, 240] = wkT[0:128, :].sum(axis=1)
    wkt[0:112, 1, 240] = wkT[128:240, :].sum(axis=1)
    wvf = np.asarray(wv).astype(np.float32)
    wvs = np.zeros((128, 2, 240), np.float32)
    wvs[:, 0, :] = wvf[0:128, :]
    wvs[0:112, 1, :] = wvf[128:240, :]
    w4t = np.asarray(wout[3]).T.astype(np.float32)
    w4pk = np.stack([w4t[h * 128:(h + 1) * 128, :] for h in range(H)], axis=1)
    w3t = np.asarray(wout[2]).T.astype(np.float32)
    w3pk = np.stack([w3t[0:128, :], w3t[128:256, :]], axis=1)
    w2pk = np.asarray(wout[1]).T.astype(np.float32)
    w1t = np.asarray(wout[0]).T.astype(np.float32)
    wcd = np.zeros((2, 128, 192), np.float32)
    for h in range(H):
        t, o = divmod(h, 2)
        wcd[t, o * 64 + 0:o * 64 + 32, 64:192] = w2pk[h * 32:(h + 1) * 32, :]
        wcd[t, o * 64 + 32:o * 64 + 48, 0:64] = w1t[h * 16:(h + 1) * 16, :]
    wallh = np.zeros((128, 4546), np.float32)
    wallh[:, 0:128] = np.eye(128, dtype=np.float32)
    wallh[:, 128:610] = wkt.reshape(128, 482)
    wallh[:, 610:1090] = wvs.reshape(128, 480)
    wallh[:, 1090:3138] = w4pk.reshape(128, 2048)
    wallh[:, 3138:3650] = w3pk.reshape(128, 512)
    wallh[:, 3650:3842] = wcd[0]
    wallh[:, 3842:4034] = wcd[1]
    return dict(wall=wallh.astype(BF))


def _host_consts():
    # indb pre-scaled by 1/(nblk*240) so the indicator matmul yields means
    indb = np.zeros((128, 8), np.float32)
    indb[:, 0] = 1.0 / (128 * 240)
    indb[0:64, 5] = 1.0 / (64 * 240)
    indb[64:96, 6] = 1.0 / (32 * 240)
    indb[96:112, 7] = 1.0 / (16 * 240)
    indc = np.zeros((4, 240), np.float32)
    indc[0, 0:128] = 1.0
    indc[1, 128:192] = 1.0
    indc[2, 192:224] = 1.0
    indc[3, 224:240] = 1.0
    return dict(indb=indb, indc=indc)


def _in_map(inputs, b, side, wside, consts):
    eqp, kvp = _host_pack(inputs, b, side)
    return dict(eqp=eqp, kvp=kvp, **wside[side], **consts)


def _unpack_out(raw):
    o = np.asarray(raw).reshape(128, NT, 960).transpose(1, 0, 2)
    return np.ascontiguousarray(o).reshape(N, 960).astype(np.float32)


def kernel(**inputs):
    inputs = {k: np.asarray(v, dtype=np.float32) for k, v in inputs.items()}
    if "nc" not in _CACHE:
        _CACHE["nc"] = _build_bass()
    nc = _CACHE["nc"]
    consts = _host_consts()
    wside = [_host_weights(inputs, 0), _host_weights(inputs, 1)]
    in_maps = [_in_map(inputs, core // 2, core % 2, wside, consts)
               for core in range(8)]
    res = run_bass_kernel_spmd(nc, in_maps, list(range(8)))
    out = np.empty((B, N, 2 * KV), np.float32)
    for core in range(8):
        b, side = core // 2, core % 2
        out[b, :, side * 960:(side + 1) * 960] = _unpack_out(res.results[core]["outp"])
    return out



# revision 3
# speedup vs baseline: 1.0231x; 1.0231x over previous
"""Trainium2 Bass kernel for nn_Attention_org_cross (cross-modal channel attention).

Sharding: 8 cores = 4 batches x 2 modality directions (pure data parallel).
Core (b, side=0): optical queries attend to DSM K/V -> out[b,:,0:960]; side=1 reverse.

v2 data path:
  ph1: eq/kv streamed as fp8-e3m4 (host power-of-2 scaled; inorm makes S
       scale-invariant) -> Ut[h] in PSUM.  No PE transposes: kvT comes
       pre-transposed from DRAM as e4m3 hi/lo pairs.
  ph2: S = Ut^T wkt (col 240 = row-sums); per-scale var -> r; exp(r*s);
       pr *= 1/esum (softmax fold); PE-transpose; P2 = (wv*g) @ pt;
       evac P2 as e4m3 hi/lo.
  ph3: ctxT = P2 @ kvT via fp8 DoubleRow hi/lo (3 products, lo*lo dropped);
       s4-part evac as e4m3 hi/lo (t4), s3/s2/s1 bf16 (tc1 -> b/cd repack).
  ph4: s4 out = t4 @ (w4*gw) DoubleRow hi/lo; w3/w2/w1 bf16; ost evac
       applies 1/(g*gw) resp 1/g.
"""
import sys

sys.path.insert(0, "/opt/trn_rl_repo")

import numpy as np
import ml_dtypes

import concourse.bacc as bacc
import concourse.mybir as mybir
import concourse.tile as tile
from concourse.bass_utils import run_bass_kernel_spmd

F32 = mybir.dt.float32
BF16 = mybir.dt.bfloat16
E4 = mybir.dt.float8e4
E3 = mybir.dt.float8e3
BF = ml_dtypes.bfloat16
E4NP = ml_dtypes.float8_e4m3
E3NP = ml_dtypes.float8_e3m4
MULT = mybir.AluOpType.mult
SUB = mybir.AluOpType.subtract
AExp = mybir.ActivationFunctionType.Exp
ASqrt = mybir.ActivationFunctionType.Sqrt
DR = mybir.MatmulPerfMode.DoubleRow

B, N, H, KV = 4, 4096, 4, 960
CQ = (16, 32, 64, 128)
RAW = (0, 64, 192, 448)
QOFF = (224, 192, 128, 0)        # scale i -> within-head q offset ([s4 s3 s2 s1])
KOFF = (0, 16, 48, 112)          # kv-side within-head offsets ([s1 s2 s3 s4])
EPS = 1e-5
NT = 32
NGRP = 2
NF = 8

GS = 128.0                       # wv scale (ctx = kv@G^T carries GS)
GW = 128.0                       # w4 scale
QDT = E3                         # ph1 stream dtype: E3 (fast) or BF16 (safe)

# wall offsets: idt | wkt | wvs*g | w3 | wcdA | wcdB | zz
W_IDT, W_WKT, W_WVS, W_W3, W_CDA, W_CDB, W_ZZ, W_END = \
    0, 128, 610, 1090, 1602, 1794, 1986, 2498

_CACHE = {}


def _build_bass(qdt):
    nc = bacc.Bacc(trn_type="TRN2", target_bir_lowering=False, debug=False)

    eqp = nc.declare_dram_parameter("eqp", [128, NT * 960], qdt, isOutput=False)
    kvp = nc.declare_dram_parameter("kvp", [128, NT * 960], qdt, isOutput=False)
    # kvthl: [128, f(8) hl(2) h(4) u(2) c(512)] e4m3, u1 partitions 112:128 zero
    kvthl = nc.declare_dram_parameter("kvthl", [128, 8 * 2 * 4096], E4,
                                      isOutput=False)
    wall = nc.declare_dram_parameter("wall", [128, W_END], BF16, isOutput=False)
    w4hl = nc.declare_dram_parameter("w4hl", [128, 4096], E4, isOutput=False)
    indb = nc.declare_dram_parameter("indb", [128, 8], F32, isOutput=False)
    indc = nc.declare_dram_parameter("indc", [4, 240], F32, isOutput=False)
    outp = nc.declare_dram_parameter("outp", [128, NT * 960], BF16, isOutput=True)

    eq_r = eqp.rearrange("p (t c) -> p t c", t=NT)
    kv_r = kvp.rearrange("p (t c) -> p t c", t=NT)
    kvt_r = kvthl.rearrange("p (f l c) -> p f l c", f=NF, l=2)
    out_r = outp.rearrange("p (t c) -> p t c", t=NT)

    def cp(e, out, in_):
        if e % 2 == 0:
            nc.vector.tensor_copy(out, in_)
        else:
            nc.scalar.copy(out, in_)

    def cpmul(e, out, in_, s):
        if e % 2 == 0:
            nc.vector.tensor_scalar(out=out, in0=in_, scalar1=s, scalar2=None,
                                    op0=MULT)
        else:
            nc.scalar.mul(out, in_, s)

    with tile.TileContext(nc) as tc:
        from contextlib import ExitStack
        with ExitStack() as outer:
            wts = outer.enter_context(tc.tile_pool(name="wts", bufs=1))
            p2sb = outer.enter_context(tc.tile_pool(name="p2sb", bufs=1))
            gsbp = outer.enter_context(tc.tile_pool(name="gsbp", bufs=1))
            esp = outer.enter_context(tc.tile_pool(name="esp", bufs=1))
            gstack = ExitStack()
            gbp = gstack.enter_context(tc.tile_pool(name="gbp", bufs=1, space="PSUM"))

            wall_sb = wts.tile([128, W_END], BF16, tag="wall")
            nc.sync.dma_start(out=wall_sb[:, 0:128], in_=wall[:, 0:128])
            nc.vector.memset(wall_sb[0:1, W_ZZ:W_END], 0.0)
            idt = wall_sb[:, W_IDT:W_IDT + 128]
            wkt_sb = wall_sb[:, W_WKT:W_WVS].rearrange("p (u c) -> p u c", u=2)
            wvs_sb = wall_sb[:, W_WVS:W_W3].rearrange("p (u c) -> p u c", u=2)
            w3_sb = wall_sb[:, W_W3:W_CDA].rearrange("p (u c) -> p u c", u=2)
            wcdA_sb = wall_sb[:, W_CDA:W_CDB]
            wcdB_sb = wall_sb[:, W_CDB:W_ZZ]
            zzt = wall_sb[0:1, W_ZZ:W_END]
            w4sb = wts.tile([128, 4096], E4, tag="w4sb")
            w4r = w4sb.rearrange("p (l q k c) -> p l q k c", l=2, q=2, k=2)
            indb_sb = wts.tile([128, 8], F32, tag="indb")
            indc_sb = wts.tile([4, 240], F32, tag="indc")
            eps_t = wts.tile([4, 1], F32, tag="eps")
            nc.scalar.dma_start(out=indb_sb, in_=indb[:, :])
            nc.scalar.dma_start(out=indc_sb, in_=indc[:, :])
            nc.vector.memset(eps_t, EPS)
            # preload the sqrt act table off the critical path
            sqd = wts.tile([4, 1], F32, tag="sqd")
            nc.scalar.activation(out=sqd, in_=eps_t, func=ASqrt, bias=eps_t)

            p2hi = [p2sb.tile([128, 2, 240], E4, tag=f"p2h{h}", name=f"p2h{h}")
                    for h in range(H)]
            p2lo = [p2sb.tile([128, 2, 240], E4, tag=f"p2l{h}", name=f"p2l{h}")
                    for h in range(H)]
            for h in range(H):
                # u1 garbage partitions must be finite: lhsT rows 112:128 of
                # ktile 1 multiply kvT's zero rows (host-padded).  Engine
                # accesses must start at a x32 partition, so clear 96:128
                # before the evac rewrites 96:112.
                nc.vector.memset(p2hi[h][96:128, 1, :], 0.0)
                nc.vector.memset(p2lo[h][96:128, 1, :], 0.0)
            gsb = [gsbp.tile([128, 2, 240], BF16, tag=f"gsb{h}", name=f"gsb{h}")
                   for h in range(H)]
            esr = [esp.tile([128, 2], F32, tag=f"esr{h}", name=f"esr{h}")
                   for h in range(H)]
            gb = [gbp.tile([128, 480], F32, tag=f"g{h}", name=f"g{h}")
                  for h in range(H)]

            # ---- phase 1: Ut accumulation (lhsT=kv) ----
            with tc.tile_pool(name="stream", bufs=3) as stream:
                for h in range(H):
                    nc.tensor.matmul(gb[h], zzt[0:1, 0:128], zzt[0:1, 0:480],
                                     start=True, stop=False)
                for g in range(NT // NGRP):
                    eq_t = stream.tile([128, NGRP, 960], qdt, tag="eq")
                    kv_t = stream.tile([128, NGRP, 960], qdt, tag="kv")
                    j0 = g * NGRP
                    nc.sync.dma_start(out=kv_t, in_=kv_r[:, j0:j0 + NGRP, :])
                    nc.sync.dma_start(out=eq_t, in_=eq_r[:, j0:j0 + NGRP, :])
                    if g == 8:
                        nc.scalar.dma_start(out=wall_sb[:, 128:W_ZZ],
                                            in_=wall[:, 128:W_ZZ])
                    if g == 10:
                        nc.scalar.dma_start(out=w4sb, in_=w4hl[:, :])
                    for h in range(H):
                        hq = hk = h * 240
                        for jj in range(NGRP):
                            nc.tensor.matmul(
                                gb[h][:, 0:240],
                                kv_t[:, jj, hk:hk + 128],
                                eq_t[:, jj, hq:hq + 240],
                                start=False, stop=False)
                            nc.tensor.matmul(
                                gb[h][0:112, 240:480],
                                kv_t[:, jj, hk + 128:hk + 240],
                                eq_t[:, jj, hq:hq + 240],
                                start=False, stop=False)
                for h in range(H):
                    nc.tensor.matmul(gb[h], zzt[0:1, 0:128], zzt[0:1, 0:480],
                                     start=False, stop=True)

            # ---- Ut evac, free banks ----
            for h in range(H):
                cp(h, gsb[h][:, 0, :], gb[h][:, 0:240])
                cp(h + 1, gsb[h][0:112, 1, :], gb[h][0:112, 240:480])
            gstack.close()

            # ---- phase 2: step-interleaved ----
            with tc.tile_pool(name="phw", bufs=5, space="PSUM") as phw, \
                 tc.tile_pool(name="phb", bufs=3, space="PSUM") as phb, \
                 tc.tile_pool(name="sm", bufs=1) as sm:
                scp, st0, st1, tiny, var, rall = {}, {}, {}, {}, {}, {}
                rv, pr0, pr1, sq0, sq1 = {}, {}, {}, {}, {}
                for h in range(H):
                    scp[h] = phw.tile([128, 2, 246], F32, tag="w", name=f"scp{h}")
                    tiny[h] = scp[h][:, 0, 242:246]
                    nc.tensor.matmul(scp[h][:, 0, 0:241], gsb[h][:, 0, 0:128],
                                     wkt_sb[:, 0, :], start=True, stop=False)
                    nc.tensor.matmul(scp[h][:, 0, 0:241], gsb[h][0:112, 1, 0:128],
                                     wkt_sb[0:112, 1, :], start=False, stop=True)
                    nc.tensor.matmul(scp[h][0:112, 1, 0:241], gsb[h][:, 0, 128:240],
                                     wkt_sb[:, 0, :], start=True, stop=False)
                    nc.tensor.matmul(scp[h][0:112, 1, 0:241],
                                     gsb[h][0:112, 1, 128:240],
                                     wkt_sb[0:112, 1, :], start=False, stop=True)
                for h in range(H):
                    st0[h] = sm.tile([128, 2], F32, tag=f"st0{h}", name=f"st0{h}")
                    st1[h] = sm.tile([112, 2], F32, tag=f"st1{h}", name=f"st1{h}")
                    nc.vector.tensor_copy(st0[h][:, 0:1], scp[h][:, 0, 240:241])
                    nc.vector.tensor_copy(st1[h][:, 0:1], scp[h][0:112, 1, 240:241])
                    sq0[h] = sm.tile([128, 240], BF16, tag=f"sq0{h}", name=f"sq0{h}")
                    sq1[h] = sm.tile([112, 240], BF16, tag=f"sq1{h}", name=f"sq1{h}")
                    nc.scalar.activation(out=sq0[h], in_=scp[h][:, 0, 0:240],
                                         func=mybir.ActivationFunctionType.Square,
                                         accum_out=st0[h][:, 1:2])
                    nc.scalar.activation(out=sq1[h], in_=scp[h][0:112, 1, 0:240],
                                         func=mybir.ActivationFunctionType.Square,
                                         accum_out=st1[h][:, 1:2])
                for h in range(H):
                    nc.tensor.matmul(tiny[h][0:4, 2:4], indb_sb[:, 0:4], st0[h],
                                     start=True, stop=False)
                    nc.tensor.matmul(tiny[h][0:4, 2:4], indb_sb[0:112, 4:8], st1[h],
                                     start=False, stop=True)
                for h in range(H):
                    # indb is pre-scaled by 1/(nblk*240): tiny holds (mean, meansq)
                    var[h] = sm.tile([4, 1], F32, tag=f"var{h}", name=f"var{h}")
                    nc.scalar.activation(out=var[h], in_=tiny[h][0:4, 2:3],
                                         func=mybir.ActivationFunctionType.Square)
                    nc.vector.tensor_tensor(out=var[h], in0=tiny[h][0:4, 3:4],
                                            in1=var[h], op=SUB)
                for h in range(H):
                    rall[h] = sm.tile([4, 1], F32, tag=f"rall{h}", name=f"rall{h}")
                    nc.scalar.activation(out=rall[h], in_=var[h], func=ASqrt,
                                         bias=eps_t)
                    nc.vector.reciprocal(out=rall[h], in_=rall[h])
                for h in range(H):
                    nc.tensor.matmul(tiny[h][:, 0:1], indc_sb[:, 0:128], rall[h],
                                     start=True, stop=True)
                    nc.tensor.matmul(tiny[h][0:112, 1:2], indc_sb[:, 128:240],
                                     rall[h], start=True, stop=True)
                    rv[h] = sm.tile([128, 2], F32, tag=f"rv{h}", name=f"rv{h}")
                    nc.vector.tensor_copy(rv[h][:, 0:1], tiny[h][:, 0:1])
                    nc.vector.tensor_copy(rv[h][0:112, 1:2], tiny[h][0:112, 1:2])
                for h in range(H):
                    # inorm guarantees ~unit-variance scores: shift-free softmax
                    pr0[h] = sm.tile([128, 240], BF16, tag=f"pr0{h}", name=f"pr0{h}")
                    pr1[h] = sm.tile([112, 240], BF16, tag=f"pr1{h}", name=f"pr1{h}")
                    nc.scalar.activation(out=pr0[h], in_=scp[h][:, 0, 0:240],
                                         func=AExp, scale=rv[h][:, 0:1],
                                         accum_out=esr[h][:, 0:1])
                    nc.scalar.activation(out=pr1[h], in_=scp[h][0:112, 1, 0:240],
                                         func=AExp, scale=rv[h][0:112, 1:2],
                                         accum_out=esr[h][0:112, 1:2])
                    nc.vector.reciprocal(out=esr[h][:, 0:1], in_=esr[h][:, 0:1])
                    nc.vector.reciprocal(out=esr[h][0:112, 1:2],
                                         in_=esr[h][0:112, 1:2])
                for h in range(H):
                    # fold softmax 1/esum into P before the transpose
                    nc.vector.tensor_scalar(out=pr0[h], in0=pr0[h],
                                            scalar1=esr[h][:, 0:1], scalar2=None,
                                            op0=MULT)
                    nc.gpsimd.tensor_scalar(out=pr1[h], in0=pr1[h],
                                            scalar1=esr[h][0:112, 1:2],
                                            scalar2=None, op0=MULT)
                for h in range(H):
                    ptp = phb.tile([128, 2, 240], BF16, tag="b", name=f"ptp{h}")
                    nc.tensor.transpose(ptp[:, 0, 0:128], pr0[h][:, 0:128], idt)
                    nc.tensor.transpose(ptp[0:112, 1, 0:128], pr0[h][:, 128:240],
                                        idt)
                    nc.tensor.transpose(ptp[:, 0, 128:240], pr1[h][:, 0:128],
                                        idt[0:112, 0:112])
                    nc.tensor.transpose(ptp[0:112, 1, 128:240],
                                        pr1[h][:, 128:240], idt[0:112, 0:112])
                    pt = sm.tile([128, 2, 240], BF16, tag=f"pt{h}", name=f"pt{h}")
                    cp(0, pt[:, 0, :], ptp[:, 0, :])
                    cp(1, pt[0:112, 1, :], ptp[0:112, 1, :])
                    p2p = phw.tile([128, 2, 246], F32, tag="w", name=f"p2p{h}")
                    nc.tensor.matmul(p2p[:, 0, 0:240], wvs_sb[:, 0, 0:128],
                                     pt[:, 0, :], start=True, stop=False)
                    nc.tensor.matmul(p2p[:, 0, 0:240], wvs_sb[0:112, 1, 0:128],
                                     pt[0:112, 1, :], start=False, stop=True)
                    nc.tensor.matmul(p2p[0:112, 1, 0:240], wvs_sb[:, 0, 128:240],
                                     pt[:, 0, :], start=True, stop=False)
                    nc.tensor.matmul(p2p[0:112, 1, 0:240], wvs_sb[0:112, 1, 128:240],
                                     pt[0:112, 1, :], start=False, stop=True)
                    # e4m3 hi/lo evac of G^T (scaled by GS via wvs)
                    nc.scalar.copy(p2hi[h][:, 0, :], p2p[:, 0, 0:240])
                    nc.vector.tensor_tensor(out=p2lo[h][:, 0, :],
                                            in0=p2p[:, 0, 0:240],
                                            in1=p2hi[h][:, 0, :], op=SUB)
                    nc.scalar.copy(p2hi[h][0:112, 1, :], p2p[0:112, 1, 0:240])
                    nc.vector.tensor_tensor(out=p2lo[h][0:112, 1, :],
                                            in0=p2p[0:112, 1, 0:240],
                                            in1=p2hi[h][0:112, 1, :], op=SUB)

            # ---- phases 3+4 fully fused per f-block ----
            ctg = outer.enter_context(tc.tile_pool(name="ctg", bufs=1))
            ostp = outer.enter_context(tc.tile_pool(name="ostp", bufs=3))
            kvs = outer.enter_context(tc.tile_pool(name="kvs", bufs=3))
            t4hi = ctg.tile([128, 4, N], E4, tag="t4hi")
            t4lo = ctg.tile([128, 4, N], E4, tag="t4lo")
            tc1 = [ctg.tile([112, N], BF16, tag=f"tc1{h}", name=f"tc1{h}")
                   for h in range(H)]
            b01 = ctg.tile([128, N], BF16, tag="b01")
            b23 = ctg.tile([128, N], BF16, tag="b23")
            cdA = ctg.tile([128, N], BF16, tag="cdA")
            cdB = ctg.tile([128, N], BF16, tag="cdB")
            nc.vector.memset(cdA, 0.0)
            nc.gpsimd.memset(cdB, 0.0)
            with tc.tile_pool(name="cps", bufs=3, space="PSUM") as cps, \
                 tc.tile_pool(name="cps1", bufs=2, space="PSUM") as cps1, \
                 tc.tile_pool(name="ops", bufs=3, space="PSUM") as ops:
                def ph3_block(f):
                    fc = slice(f * 512, (f + 1) * 512)
                    kh = kvs.tile([128, 4096], E4, tag="kh", name=f"kh{f}")
                    kl = kvs.tile([128, 4096], E4, tag="kl", name=f"kl{f}")
                    nc.sync.dma_start(out=kh, in_=kvt_r[:, f, 0, :])
                    nc.sync.dma_start(out=kl, in_=kvt_r[:, f, 1, :])
                    khr = kh.rearrange("p (h u c) -> p h u c", h=4, u=2)
                    klr = kl.rearrange("p (h u c) -> p h u c", h=4, u=2)
                    for h in range(H):
                        c0 = cps.tile([128, 512], F32, tag="c0")
                        c1 = cps1.tile([112, 512], F32, tag="c1")
                        a0h = p2hi[h][:, :, 0:128]
                        a0l = p2lo[h][:, :, 0:128]
                        a1h = p2hi[h][:, :, 128:240]
                        a1l = p2lo[h][:, :, 128:240]
                        bh, bl = khr[:, h, :, :], klr[:, h, :, :]
                        nc.tensor.matmul(c0, a0h, bh, start=True, stop=False,
                                         perf_mode=DR)
                        nc.tensor.matmul(c0, a0l, bh, start=False, stop=False,
                                         perf_mode=DR)
                        nc.tensor.matmul(c0, a0h, bl, start=False, stop=True,
                                         perf_mode=DR)
                        nc.tensor.matmul(c1, a1h, bh, start=True, stop=False,
                                         perf_mode=DR)
                        nc.tensor.matmul(c1, a1l, bh, start=False, stop=False,
                                         perf_mode=DR)
                        nc.tensor.matmul(c1, a1h, bl, start=False, stop=True,
                                         perf_mode=DR)
                        o = (h % 2) * 64
                        bt = b01 if h < 2 else b23
                        cd = cdA if h < 2 else cdB
                        e = (h + f) % 2
                        cp(e, t4hi[:, h, fc], c0)
                        nc.vector.tensor_tensor(out=t4lo[:, h, fc], in0=c0,
                                                in1=t4hi[:, h, fc], op=SUB)
                        cp(e + 1, tc1[h][:, fc], c1)
                        nc.gpsimd.tensor_copy(bt[o:o + 64, fc], tc1[h][0:64, fc])
                        nc.gpsimd.tensor_copy(cd[o:o + 48, fc], tc1[h][64:112, fc])

                def ph4_block(f, interleave=False):
                    ost = ostp.tile([128, 4, 960], BF16, tag="ost")
                    def bp_block(q):
                        j = 4 * f + q
                        ncol = slice(j * 128, (j + 1) * 128)
                        bp = ops.tile([128, 512], F32, tag="o", name=f"bp{j}")
                        for p in range(2):
                            hh = slice(2 * p, 2 * p + 2)
                            nc.tensor.matmul(bp, t4hi[:, hh, ncol],
                                             w4r[:, 0, p, :, :],
                                             start=(p == 0), stop=False,
                                             perf_mode=DR)
                            nc.tensor.matmul(bp, t4lo[:, hh, ncol],
                                             w4r[:, 0, p, :, :],
                                             start=False, stop=False,
                                             perf_mode=DR)
                            nc.tensor.matmul(bp, t4hi[:, hh, ncol],
                                             w4r[:, 1, p, :, :],
                                             start=False, stop=(p == 1),
                                             perf_mode=DR)
                        cpmul(q, ost[:, q, 448:960], bp, 1.0 / (GS * GW))
                    def ap_block(q, dma_pair):
                        j = 4 * f + q
                        ncol = slice(j * 128, (j + 1) * 128)
                        ap = ops.tile([128, 512], F32, tag="o", name=f"ap{j}")
                        nc.tensor.matmul(ap[:, 192:448], b01[:, ncol],
                                         w3_sb[:, 0, :], start=True, stop=False)
                        nc.tensor.matmul(ap[:, 192:448], b23[:, ncol],
                                         w3_sb[:, 1, :], start=False, stop=True)
                        nc.tensor.matmul(ap[:, 0:192], cdA[:, ncol], wcdA_sb,
                                         start=True, stop=False)
                        nc.tensor.matmul(ap[:, 0:192], cdB[:, ncol], wcdB_sb,
                                         start=False, stop=True)
                        cpmul(q + 1, ost[:, q, 0:448], ap[:, 0:448], 1.0 / GS)
                        if not dma_pair:
                            nc.sync.dma_start(out=out_r[:, j:j + 1, :],
                                              in_=ost[:, q:q + 1, :])
                        elif q % 2 == 1:
                            nc.sync.dma_start(
                                out=out_r[:, j - 1:j + 1, :],
                                in_=ost[:, q - 1:q + 1, :])
                    if interleave:
                        for q in range(4):
                            bp_block(q)
                            ap_block(q, dma_pair=False)
                    else:
                        for q in range(4):
                            bp_block(q)
                        for q in range(4):
                            ap_block(q, dma_pair=(f != NF - 1))

                # software-pipelined by one f-block: ph4 consumes f-1 while
                # ph3 produces f
                for step in range(NF + 1):
                    if step < NF:
                        ph3_block(step)
                    if step >= 1:
                        ph4_block(step - 1, interleave=(step == NF))
    nc.finalize()
    return nc


def _host_pack(inputs, b, side, qdt):
    if side == 0:
        embs = [inputs['emb1'], inputs['emb2'], inputs['emb3'], inputs['emb4']]
        wq = [inputs[f'wq{i+1}'] for i in range(4)]
        kvsrc = inputs['emb_alld']
    else:
        embs = [inputs['embd1'], inputs['embd2'], inputs['embd3'], inputs['embd4']]
        wq = [inputs[f'wqd{i+1}'] for i in range(4)]
        kvsrc = inputs['emb_all']
    scale = np.float32(1.0 / np.sqrt(np.float32(KV)))
    eq = np.empty((N, 960), np.float32)
    kvf = np.empty((N, 960), np.float32)
    for h in range(H):
        for i in range(4):
            cq = CQ[i]
            blkq = np.asarray(embs[i][b][:, h * cq:(h + 1) * cq], np.float32)
            # fold wq (and 1/sqrt(KV)) into the q-side pack: Q = emb @ wq^T
            eq[:, h * 240 + QOFF[i]: h * 240 + QOFF[i] + cq] = \
                (blkq @ np.asarray(wq[i][h], np.float32).T) * scale
            kvf[:, h * 240 + KOFF[i]: h * 240 + KOFF[i] + cq] = \
                kvsrc[b][:, RAW[i] + h * cq: RAW[i] + (h + 1) * cq]

    # kvT e4m3 hi/lo pack: [128, f hl h u c], u1 partitions 112:128 zero
    kvt = np.zeros((128, NF, 2, H, 2, 512), np.float32)
    for h in range(H):
        kT = kvf[:, h * 240:(h + 1) * 240].T  # (240, N)
        for u, (r0, r1) in enumerate(((0, 128), (128, 240))):
            blk = kT[r0:r1].reshape(r1 - r0, NF, 512)
            hi = blk.astype(E4NP).astype(np.float32)
            lo = (blk - hi).astype(E4NP).astype(np.float32)
            kvt[0:r1 - r0, :, 0, h, u, :] = hi.transpose(0, 1, 2)
            kvt[0:r1 - r0, :, 1, h, u, :] = lo
    kvthl = np.ascontiguousarray(kvt.reshape(128, -1)).astype(E4NP)

    if qdt is E3:
        # per-head power-of-2 scaling into the e3m4 sweet spot (absmax ~12);
        # inorm makes S invariant to any per-head constant factor
        for h in range(H):
            cs = slice(h * 240, (h + 1) * 240)
            eq[:, cs] *= 2.0 ** np.floor(np.log2(12.0 / np.abs(eq[:, cs]).max()))
            kvf[:, cs] *= 2.0 ** np.floor(np.log2(12.0 / np.abs(kvf[:, cs]).max()))
        qnp = E3NP
    else:
        qnp = BF
    eqp = np.ascontiguousarray(
        eq.reshape(NT, 128, 960).transpose(1, 0, 2)).reshape(128, NT * 960)
    kvp = np.ascontiguousarray(
        kvf.reshape(NT, 128, 960).transpose(1, 0, 2)).reshape(128, NT * 960)
    return eqp.astype(qnp), kvp.astype(qnp), kvthl


def _host_weights(inputs, side):
    if side == 0:
        wk, wv = inputs['wkd'], inputs['wvd']
        wout = [inputs[f'wout{i+1}'] for i in range(4)]
    else:
        wk, wv = inputs['wk'], inputs['wv']
        wout = [inputs[f'woutd{i+1}'] for i in range(4)]
    wkT = np.asarray(wk).T.astype(np.float32)
    wkt = np.zeros((128, 2, 241), np.float32)
    wkt[:, 0, 0:240] = wkT[0:128, :]
    wkt[0:112, 1, 0:240] = wkT[128:240, :]
    wkt[:, 0, 240] = wkT[0:128, :].sum(axis=1)
    wkt[0:112, 1, 240] = wkT[128:240, :].sum(axis=1)
    wvf = np.asarray(wv).astype(np.float32) * np.float32(GS)
    wvs = np.zeros((128, 2, 240), np.float32)
    wvs[:, 0, :] = wvf[0:128, :]
    wvs[0:112, 1, :] = wvf[128:240, :]
    w3t = np.asarray(wout[2]).T.astype(np.float32)
    w3pk = np.stack([w3t[0:128, :], w3t[128:256, :]], axis=1)
    w2pk = np.asarray(wout[1]).T.astype(np.float32)
    w1t = np.asarray(wout[0]).T.astype(np.float32)
    wcd = np.zeros((2, 128, 192), np.float32)
    for h in range(H):
        t, o = divmod(h, 2)
        wcd[t, o * 64 + 0:o * 64 + 32, 64:192] = w2pk[h * 32:(h + 1) * 32, :]
        wcd[t, o * 64 + 32:o * 64 + 48, 0:64] = w1t[h * 16:(h + 1) * 16, :]
    wallh = np.zeros((128, W_END), np.float32)
    wallh[:, 0:128] = np.eye(128, dtype=np.float32)
    wallh[:, W_WKT:W_WVS] = wkt.reshape(128, 482)
    wallh[:, W_WVS:W_W3] = wvs.reshape(128, 480)
    wallh[:, W_W3:W_CDA] = w3pk.reshape(128, 512)
    wallh[:, W_CDA:W_CDB] = wcd[0]
    wallh[:, W_CDB:W_ZZ] = wcd[1]

    # w4 * GW as e4m3 hi/lo: [128, hl pair kt c]
    w4t = np.asarray(wout[3]).T.astype(np.float32) * np.float32(GW)
    w4pk = np.stack([w4t[h * 128:(h + 1) * 128, :] for h in range(H)], axis=1)
    w4a = np.zeros((128, 2, 2, 2, 512), np.float32)
    for p in range(2):
        for k in range(2):
            blk = w4pk[:, 2 * p + k, :]
            hi = blk.astype(E4NP).astype(np.float32)
            w4a[:, 0, p, k, :] = hi
            w4a[:, 1, p, k, :] = (blk - hi).astype(E4NP).astype(np.float32)
    w4hl = np.ascontiguousarray(w4a.reshape(128, 4096)).astype(E4NP)
    return dict(wall=wallh.astype(BF), w4hl=w4hl)


def _host_consts():
    # indb pre-scaled by 1/(nblk*240) so the indicator matmul yields means
    indb = np.zeros((128, 8), np.float32)
    indb[:, 0] = 1.0 / (128 * 240)
    indb[0:64, 5] = 1.0 / (64 * 240)
    indb[64:96, 6] = 1.0 / (32 * 240)
    indb[96:112, 7] = 1.0 / (16 * 240)
    indc = np.zeros((4, 240), np.float32)
    indc[0, 0:128] = 1.0
    indc[1, 128:192] = 1.0
    indc[2, 192:224] = 1.0
    indc[3, 224:240] = 1.0
    return dict(indb=indb, indc=indc)


def _in_map(inputs, b, side, wside, consts):
    eqp, kvp, kvthl = _host_pack(inputs, b, side, QDT)
    return dict(eqp=eqp, kvp=kvp, kvthl=kvthl, **wside[side], **consts)


def _unpack_out(raw):
    o = np.asarray(raw).reshape(128, NT, 960).transpose(1, 0, 2)
    return np.ascontiguousarray(o).reshape(N, 960).astype(np.float32)


def kernel(**inputs):
    inputs = {k: np.asarray(v, dtype=np.float32) for k, v in inputs.items()}
    key = ("nc", str(QDT))
    if key not in _CACHE:
        _CACHE[key] = _build_bass(QDT)
        _CACHE["nc"] = _CACHE[key]
    nc = _CACHE[key]
    consts = _host_consts()
    wside = [_host_weights(inputs, 0), _host_weights(inputs, 1)]
    in_maps = [_in_map(inputs, core // 2, core % 2, wside, consts)
               for core in range(8)]
    res = run_bass_kernel_spmd(nc, in_maps, list(range(8)))
    out = np.empty((B, N, 2 * KV), np.float32)
    for core in range(8):
        b, side = core // 2, core % 2
        out[b, :, side * 960:(side + 1) * 960] = _unpack_out(res.results[core]["outp"])
    return out


# revision 6
# speedup vs baseline: 1.1424x; 1.1166x over previous
"""Trainium2 Bass kernel for nn_Attention_org_cross (cross-modal channel attention).

Sharding: 8 cores = 4 batches x 2 modality directions (pure data parallel).
Core (b, side=0): optical queries attend to DSM K/V -> out[b,:,0:960]; side=1 reverse.

v2 data path:
  ph1: eq/kv streamed as fp8-e3m4 (host power-of-2 scaled; inorm makes S
       scale-invariant) -> Ut[h] in PSUM.  No PE transposes: kvT comes
       pre-transposed from DRAM as e4m3 hi/lo pairs.
  ph2: S = Ut^T wkt (col 240 = row-sums); per-scale var -> r; exp(r*s);
       pr *= 1/esum (softmax fold); PE-transpose; P2 = (wv*g) @ pt;
       evac P2 as e4m3 hi/lo.
  ph3: ctxT = P2 @ kvT via fp8 DoubleRow hi/lo (3 products, lo*lo dropped);
       s4-part evac as e4m3 hi/lo (t4), s3/s2/s1 bf16 (tc1 -> b/cd repack).
  ph4: s4 out = t4 @ (w4*gw) DoubleRow hi/lo; w3/w2/w1 bf16; ost evac
       applies 1/(g*gw) resp 1/g.
"""
import sys

sys.path.insert(0, "/opt/trn_rl_repo")

import numpy as np
import ml_dtypes

import concourse.bacc as bacc
import concourse.mybir as mybir
import concourse.tile as tile
from concourse.bass_utils import run_bass_kernel_spmd

F32 = mybir.dt.float32
BF16 = mybir.dt.bfloat16
E4 = mybir.dt.float8e4
E3 = mybir.dt.float8e3
BF = ml_dtypes.bfloat16
E4NP = ml_dtypes.float8_e4m3
E3NP = ml_dtypes.float8_e3m4
MULT = mybir.AluOpType.mult
SUB = mybir.AluOpType.subtract
AExp = mybir.ActivationFunctionType.Exp
ASqrt = mybir.ActivationFunctionType.Sqrt
DR = mybir.MatmulPerfMode.DoubleRow

B, N, H, KV = 4, 4096, 4, 960
CQ = (16, 32, 64, 128)
RAW = (0, 64, 192, 448)
QOFF = (224, 192, 128, 0)        # scale i -> within-head q offset ([s4 s3 s2 s1])
KOFF = (0, 16, 48, 112)          # kv-side within-head offsets ([s1 s2 s3 s4])
EPS = 1e-5
NT = 32
NGRP = 2
NF = 8

GS = 128.0                       # wv scale (ctx = kv@G^T carries GS)
GW = 128.0                       # w4 scale
QDT = E3                         # ph1 stream dtype: E3 (fast) or BF16 (safe)

# wall offsets: idt | wkt | wvs*g | w3 | wcdA | wcdB | zz
W_IDT, W_WKT, W_WVS, W_W3, W_CDA, W_CDB, W_ZZ, W_END = \
    0, 128, 610, 1090, 1602, 1794, 1986, 2498

_CACHE = {}


def _build_bass(qdt):
    nc = bacc.Bacc(trn_type="TRN2", target_bir_lowering=False, debug=False)

    eqp = nc.declare_dram_parameter("eqp", [128, NT * 960], qdt, isOutput=False)
    kvp = nc.declare_dram_parameter("kvp", [128, NT * 960], qdt, isOutput=False)
    # kvthl: [128, f(8) hl(2) h(4) u(2) c(512)] e4m3, u1 partitions 112:128 zero
    kvthl = nc.declare_dram_parameter("kvthl", [128, 8 * 2 * 4096], E4,
                                      isOutput=False)
    wall = nc.declare_dram_parameter("wall", [128, W_END], BF16, isOutput=False)
    w4hl = nc.declare_dram_parameter("w4hl", [128, 4096], E4, isOutput=False)
    indb = nc.declare_dram_parameter("indb", [128, 8], F32, isOutput=False)
    indc = nc.declare_dram_parameter("indc", [4, 240], F32, isOutput=False)
    outp = nc.declare_dram_parameter("outp", [128, NT * 960], BF16, isOutput=True)

    eq_r = eqp.rearrange("p (t c) -> p t c", t=NT)
    kv_r = kvp.rearrange("p (t c) -> p t c", t=NT)
    kvt_r = kvthl.rearrange("p (f l c) -> p f l c", f=NF, l=2)
    out_r = outp.rearrange("p (t c) -> p t c", t=NT)

    def cp(e, out, in_):
        if e % 2 == 0:
            nc.vector.tensor_copy(out, in_)
        else:
            nc.scalar.copy(out, in_)

    def cpmul(e, out, in_, s):
        if e % 2 == 0:
            nc.vector.tensor_scalar(out=out, in0=in_, scalar1=s, scalar2=None,
                                    op0=MULT)
        else:
            nc.scalar.mul(out, in_, s)

    with tile.TileContext(nc) as tc:
        from contextlib import ExitStack
        with ExitStack() as outer:
            wts = outer.enter_context(tc.tile_pool(name="wts", bufs=1))
            p2sb = outer.enter_context(tc.tile_pool(name="p2sb", bufs=1))
            gsbp = outer.enter_context(tc.tile_pool(name="gsbp", bufs=1))
            esp = outer.enter_context(tc.tile_pool(name="esp", bufs=1))
            gstack = ExitStack()
            gbp = gstack.enter_context(tc.tile_pool(name="gbp", bufs=1, space="PSUM"))

            wall_sb = wts.tile([128, W_END], BF16, tag="wall")
            nc.sync.dma_start(out=wall_sb[:, 0:128], in_=wall[:, 0:128])
            nc.vector.memset(wall_sb[0:1, W_ZZ:W_END], 0.0)
            idt = wall_sb[:, W_IDT:W_IDT + 128]
            wkt_sb = wall_sb[:, W_WKT:W_WVS].rearrange("p (u c) -> p u c", u=2)
            wvs_sb = wall_sb[:, W_WVS:W_W3].rearrange("p (u c) -> p u c", u=2)
            w3_sb = wall_sb[:, W_W3:W_CDA].rearrange("p (u c) -> p u c", u=2)
            wcdA_sb = wall_sb[:, W_CDA:W_CDB]
            wcdB_sb = wall_sb[:, W_CDB:W_ZZ]
            zzt = wall_sb[0:1, W_ZZ:W_END]
            w4sb = wts.tile([128, 4096], E4, tag="w4sb")
            w4r = w4sb.rearrange("p (l q k c) -> p l q k c", l=2, q=2, k=2)
            indb_sb = wts.tile([128, 8], F32, tag="indb")
            indc_sb = wts.tile([4, 240], F32, tag="indc")
            eps_t = wts.tile([4, 1], F32, tag="eps")
            nc.scalar.dma_start(out=indb_sb, in_=indb[:, :])
            nc.scalar.dma_start(out=indc_sb, in_=indc[:, :])
            nc.vector.memset(eps_t, EPS)
            # preload the sqrt act table off the critical path
            sqd = wts.tile([4, 1], F32, tag="sqd")
            nc.scalar.activation(out=sqd, in_=eps_t, func=ASqrt, bias=eps_t)

            p2hi = [p2sb.tile([128, 2, 240], E4, tag=f"p2h{h}", name=f"p2h{h}")
                    for h in range(H)]
            p2lo = [p2sb.tile([128, 2, 240], E4, tag=f"p2l{h}", name=f"p2l{h}")
                    for h in range(H)]
            for h in range(H):
                # u1 garbage partitions must be finite: lhsT rows 112:128 of
                # ktile 1 multiply kvT's zero rows (host-padded).  Engine
                # accesses must start at a x32 partition, so clear 96:128
                # before the evac rewrites 96:112.
                nc.vector.memset(p2hi[h][96:128, 1, :], 0.0)
                nc.vector.memset(p2lo[h][96:128, 1, :], 0.0)
            gsb = [gsbp.tile([128, 2, 240], BF16, tag=f"gsb{h}", name=f"gsb{h}")
                   for h in range(H)]
            esr = [esp.tile([128, 2], F32, tag=f"esr{h}", name=f"esr{h}")
                   for h in range(H)]
            gb = [gbp.tile([128, 480], F32, tag=f"g{h}", name=f"g{h}")
                  for h in range(H)]

            # ---- phase 1: Ut accumulation (lhsT=kv) ----
            with tc.tile_pool(name="stream", bufs=3) as stream:
                for h in range(H):
                    nc.tensor.matmul(gb[h], zzt[0:1, 0:128], zzt[0:1, 0:480],
                                     start=True, stop=False)
                for g in range(NT // NGRP):
                    eq_t = stream.tile([128, NGRP, 960], qdt, tag="eq")
                    kv_t = stream.tile([128, NGRP, 960], qdt, tag="kv")
                    j0 = g * NGRP
                    nc.sync.dma_start(out=kv_t, in_=kv_r[:, j0:j0 + NGRP, :])
                    nc.sync.dma_start(out=eq_t, in_=eq_r[:, j0:j0 + NGRP, :])
                    if g == 8:
                        nc.scalar.dma_start(out=wall_sb[:, 128:W_ZZ],
                                            in_=wall[:, 128:W_ZZ])
                    if g == 10:
                        nc.scalar.dma_start(out=w4sb, in_=w4hl[:, :])
                    for h in range(H):
                        hq = hk = h * 240
                        for jj in range(NGRP):
                            nc.tensor.matmul(
                                gb[h][:, 0:240],
                                kv_t[:, jj, hk:hk + 128],
                                eq_t[:, jj, hq:hq + 240],
                                start=False, stop=False)
                            nc.tensor.matmul(
                                gb[h][0:112, 240:480],
                                kv_t[:, jj, hk + 128:hk + 240],
                                eq_t[:, jj, hq:hq + 240],
                                start=False, stop=False)
                for h in range(H):
                    nc.tensor.matmul(gb[h], zzt[0:1, 0:128], zzt[0:1, 0:480],
                                     start=False, stop=True)

            # ---- Ut evac, free banks ----
            for h in range(H):
                cp(h, gsb[h][:, 0, :], gb[h][:, 0:240])
                cp(h + 1, gsb[h][0:112, 1, :], gb[h][0:112, 240:480])
            gstack.close()

            # ---- phase 3/4 static tiles (allocated early so ph3(f0) can
            # run fused into phase 2) ----
            ctg = outer.enter_context(tc.tile_pool(name="ctg", bufs=1))
            ostp = outer.enter_context(tc.tile_pool(name="ostp", bufs=3))
            kvs = outer.enter_context(tc.tile_pool(name="kvs", bufs=3))
            t4hi = ctg.tile([128, 4, N], E4, tag="t4hi")
            t4lo = ctg.tile([128, 4, N], E4, tag="t4lo")
            tc1 = [ctg.tile([112, N], BF16, tag=f"tc1{h}", name=f"tc1{h}")
                   for h in range(H)]
            b01 = ctg.tile([128, N], BF16, tag="b01")
            b23 = ctg.tile([128, N], BF16, tag="b23")
            cdA = ctg.tile([128, N], BF16, tag="cdA")
            cdB = ctg.tile([128, N], BF16, tag="cdB")
            nc.vector.memset(cdA, 0.0)
            nc.gpsimd.memset(cdB, 0.0)

            def kvt_load(f, gate):
                kh = kvs.tile([128, 4096], E4, tag="kh", name=f"kh{f}")
                kl = kvs.tile([128, 4096], E4, tag="kl", name=f"kl{f}")
                if gate:
                    # order the DMA behind phase 1 (gsb is written at its
                    # end) so the transfer cannot steal ph1 stream bandwidth
                    nc.scalar.copy(kh[:, 0:1], gsb[0][:, 0, 0:1])
                    nc.scalar.copy(kl[:, 0:1], gsb[1][:, 0, 0:1])
                nc.sync.dma_start(out=kh, in_=kvt_r[:, f, 0, :])
                nc.sync.dma_start(out=kl, in_=kvt_r[:, f, 1, :])
                return (kh.rearrange("p (h u c) -> p h u c", h=4, u=2),
                        kl.rearrange("p (h u c) -> p h u c", h=4, u=2))

            def ph3_head(f, h, khr, klr, c0, c1):
                fc = slice(f * 512, (f + 1) * 512)
                a0h = p2hi[h][:, :, 0:128]
                a0l = p2lo[h][:, :, 0:128]
                a1h = p2hi[h][:, :, 128:240]
                a1l = p2lo[h][:, :, 128:240]
                bh, bl = khr[:, h, :, :], klr[:, h, :, :]
                nc.tensor.matmul(c0, a0h, bh, start=True, stop=False,
                                 perf_mode=DR)
                nc.tensor.matmul(c0, a0l, bh, start=False, stop=False,
                                 perf_mode=DR)
                nc.tensor.matmul(c0, a0h, bl, start=False, stop=True,
                                 perf_mode=DR)
                nc.tensor.matmul(c1, a1h, bh, start=True, stop=False,
                                 perf_mode=DR)
                nc.tensor.matmul(c1, a1l, bh, start=False, stop=False,
                                 perf_mode=DR)
                nc.tensor.matmul(c1, a1h, bl, start=False, stop=True,
                                 perf_mode=DR)
                o = (h % 2) * 64
                bt = b01 if h < 2 else b23
                cd = cdA if h < 2 else cdB
                nc.scalar.copy(t4hi[:, h, fc], c0)
                nc.vector.tensor_tensor(out=t4lo[:, h, fc], in0=c0,
                                        in1=t4hi[:, h, fc], op=SUB)
                cp(h + f, tc1[h][:, fc], c1)
                nc.gpsimd.tensor_copy(bt[o:o + 64, fc], tc1[h][0:64, fc])
                nc.gpsimd.tensor_copy(cd[o:o + 48, fc], tc1[h][64:112, fc])

            # ---- phase 2: step-interleaved, with ph3(f0) fused in ----
            khr0, klr0 = kvt_load(0, gate=False)
            with tc.tile_pool(name="phw", bufs=6, space="PSUM") as phw, \
                 tc.tile_pool(name="phb", bufs=2, space="PSUM") as phb, \
                 tc.tile_pool(name="sm", bufs=1) as sm:
                scp, st0, st1, tiny, var, rall = {}, {}, {}, {}, {}, {}
                rv, pr0, pr1, sq0, sq1 = {}, {}, {}, {}, {}
                for h in range(H):
                    scp[h] = phw.tile([128, 2, 246], F32, tag="w", name=f"scp{h}")
                    tiny[h] = scp[h][:, 0, 242:246]
                    nc.tensor.matmul(scp[h][:, 0, 0:241], gsb[h][:, 0, 0:128],
                                     wkt_sb[:, 0, :], start=True, stop=False)
                    nc.tensor.matmul(scp[h][:, 0, 0:241], gsb[h][0:112, 1, 0:128],
                                     wkt_sb[0:112, 1, :], start=False, stop=True)
                    nc.tensor.matmul(scp[h][0:112, 1, 0:241], gsb[h][:, 0, 128:240],
                                     wkt_sb[:, 0, :], start=True, stop=False)
                    nc.tensor.matmul(scp[h][0:112, 1, 0:241],
                                     gsb[h][0:112, 1, 128:240],
                                     wkt_sb[0:112, 1, :], start=False, stop=True)
                for h in range(H):
                    st0[h] = sm.tile([128, 2], F32, tag=f"st0{h}", name=f"st0{h}")
                    st1[h] = sm.tile([112, 2], F32, tag=f"st1{h}", name=f"st1{h}")
                    nc.vector.tensor_copy(st0[h][:, 0:1], scp[h][:, 0, 240:241])
                    nc.vector.tensor_copy(st1[h][:, 0:1], scp[h][0:112, 1, 240:241])
                    sq0[h] = sm.tile([128, 240], BF16, tag=f"sq0{h}", name=f"sq0{h}")
                    sq1[h] = sm.tile([112, 240], BF16, tag=f"sq1{h}", name=f"sq1{h}")
                    nc.scalar.activation(out=sq0[h], in_=scp[h][:, 0, 0:240],
                                         func=mybir.ActivationFunctionType.Square,
                                         accum_out=st0[h][:, 1:2])
                    nc.scalar.activation(out=sq1[h], in_=scp[h][0:112, 1, 0:240],
                                         func=mybir.ActivationFunctionType.Square,
                                         accum_out=st1[h][:, 1:2])
                for h in range(H):
                    nc.tensor.matmul(tiny[h][0:4, 2:4], indb_sb[:, 0:4], st0[h],
                                     start=True, stop=False)
                    nc.tensor.matmul(tiny[h][0:4, 2:4], indb_sb[0:112, 4:8], st1[h],
                                     start=False, stop=True)
                for h in range(H):
                    # indb is pre-scaled by 1/(nblk*240): tiny holds (mean, meansq)
                    var[h] = sm.tile([4, 1], F32, tag=f"var{h}", name=f"var{h}")
                    nc.scalar.activation(out=var[h], in_=tiny[h][0:4, 2:3],
                                         func=mybir.ActivationFunctionType.Square)
                    nc.vector.tensor_tensor(out=var[h], in0=tiny[h][0:4, 3:4],
                                            in1=var[h], op=SUB)
                for h in range(H):
                    rall[h] = sm.tile([4, 1], F32, tag=f"rall{h}", name=f"rall{h}")
                    nc.scalar.activation(out=rall[h], in_=var[h], func=ASqrt,
                                         bias=eps_t)
                    nc.vector.reciprocal(out=rall[h], in_=rall[h])
                for h in range(H):
                    nc.tensor.matmul(tiny[h][:, 0:1], indc_sb[:, 0:128], rall[h],
                                     start=True, stop=True)
                    nc.tensor.matmul(tiny[h][0:112, 1:2], indc_sb[:, 128:240],
                                     rall[h], start=True, stop=True)
                    rv[h] = sm.tile([128, 2], F32, tag=f"rv{h}", name=f"rv{h}")
                    nc.vector.tensor_copy(rv[h][:, 0:1], tiny[h][:, 0:1])
                    nc.vector.tensor_copy(rv[h][0:112, 1:2], tiny[h][0:112, 1:2])
                for h in range(H):
                    # inorm guarantees ~unit-variance scores: shift-free softmax
                    pr0[h] = sm.tile([128, 240], BF16, tag=f"pr0{h}", name=f"pr0{h}")
                    pr1[h] = sm.tile([112, 240], BF16, tag=f"pr1{h}", name=f"pr1{h}")
                    nc.scalar.activation(out=pr0[h], in_=scp[h][:, 0, 0:240],
                                         func=AExp, scale=rv[h][:, 0:1],
                                         accum_out=esr[h][:, 0:1])
                    nc.scalar.activation(out=pr1[h], in_=scp[h][0:112, 1, 0:240],
                                         func=AExp, scale=rv[h][0:112, 1:2],
                                         accum_out=esr[h][0:112, 1:2])
                    nc.vector.reciprocal(out=esr[h][:, 0:1], in_=esr[h][:, 0:1])
                    nc.vector.reciprocal(out=esr[h][0:112, 1:2],
                                         in_=esr[h][0:112, 1:2])
                for h in range(H):
                    # fold softmax 1/esum into P before the transpose
                    nc.vector.tensor_scalar(out=pr0[h], in0=pr0[h],
                                            scalar1=esr[h][:, 0:1], scalar2=None,
                                            op0=MULT)
                    nc.gpsimd.tensor_scalar(out=pr1[h], in0=pr1[h],
                                            scalar1=esr[h][0:112, 1:2],
                                            scalar2=None, op0=MULT)
                pt = {}
                for h in range(H):
                    ptp = phb.tile([128, 2, 240], BF16, tag="b", name=f"ptp{h}")
                    nc.tensor.transpose(ptp[:, 0, 0:128], pr0[h][:, 0:128], idt)
                    nc.tensor.transpose(ptp[0:112, 1, 0:128], pr0[h][:, 128:240],
                                        idt)
                    nc.tensor.transpose(ptp[:, 0, 128:240], pr1[h][:, 0:128],
                                        idt[0:112, 0:112])
                    nc.tensor.transpose(ptp[0:112, 1, 128:240],
                                        pr1[h][:, 128:240], idt[0:112, 0:112])
                    pt[h] = sm.tile([128, 2, 240], BF16, tag=f"pt{h}",
                                    name=f"pt{h}")
                    cp(0, pt[h][:, 0, :], ptp[:, 0, :])
                    cp(1, pt[h][0:112, 1, :], ptp[0:112, 1, :])
                    p2p = phw.tile([128, 2, 246], F32, tag="w", name=f"p2p{h}")
                    nc.tensor.matmul(p2p[:, 0, 0:240], wvs_sb[:, 0, 0:128],
                                     pt[h][:, 0, :], start=True, stop=False)
                    nc.tensor.matmul(p2p[:, 0, 0:240], wvs_sb[0:112, 1, 0:128],
                                     pt[h][0:112, 1, :], start=False, stop=True)
                    nc.tensor.matmul(p2p[0:112, 1, 0:240], wvs_sb[:, 0, 128:240],
                                     pt[h][:, 0, :], start=True, stop=False)
                    nc.tensor.matmul(p2p[0:112, 1, 0:240], wvs_sb[0:112, 1, 128:240],
                                     pt[h][0:112, 1, :], start=False, stop=True)
                    # e4m3 hi/lo evac of G^T (scaled by GS via wvs)
                    nc.scalar.copy(p2hi[h][:, 0, :], p2p[:, 0, 0:240])
                    nc.vector.tensor_tensor(out=p2lo[h][:, 0, :],
                                            in0=p2p[:, 0, 0:240],
                                            in1=p2hi[h][:, 0, :], op=SUB)
                    nc.scalar.copy(p2hi[h][0:112, 1, :], p2p[0:112, 1, 0:240])
                    nc.vector.tensor_tensor(out=p2lo[h][0:112, 1, :],
                                            in0=p2p[0:112, 1, 0:240],
                                            in1=p2hi[h][0:112, 1, :], op=SUB)
                    # ph3(f0) for the previous head fills PE while this
                    # head's G evac completes
                    if h > 0:
                        c0 = phw.tile([128, 512], F32, tag="w", name=f"fc0{h}")
                        c1 = phw.tile([112, 512], F32, tag="w", name=f"fc1{h}")
                        ph3_head(0, h - 1, khr0, klr0, c0, c1)
                c0 = phw.tile([128, 512], F32, tag="w", name="fc0z")
                c1 = phw.tile([112, 512], F32, tag="w", name="fc1z")
                ph3_head(0, 3, khr0, klr0, c0, c1)

            # ---- phases 3+4 fully fused per f-block ----
            with tc.tile_pool(name="cps", bufs=3, space="PSUM") as cps, \
                 tc.tile_pool(name="cps1", bufs=2, space="PSUM") as cps1, \
                 tc.tile_pool(name="ops", bufs=3, space="PSUM") as ops:
                def ph3_block(f):
                    khr, klr = kvt_load(f, gate=(f in (1, 2)))
                    for h in range(H):
                        c0 = cps.tile([128, 512], F32, tag="c0")
                        c1 = cps1.tile([112, 512], F32, tag="c1")
                        ph3_head(f, h, khr, klr, c0, c1)

                def ph4_block(f, interleave=False):
                    ost = ostp.tile([128, 4, 960], BF16, tag="ost")
                    def bp_block(q):
                        j = 4 * f + q
                        ncol = slice(j * 128, (j + 1) * 128)
                        bp = ops.tile([128, 512], F32, tag="o", name=f"bp{j}")
                        for p in range(2):
                            hh = slice(2 * p, 2 * p + 2)
                            nc.tensor.matmul(bp, t4hi[:, hh, ncol],
                                             w4r[:, 0, p, :, :],
                                             start=(p == 0), stop=False,
                                             perf_mode=DR)
                            nc.tensor.matmul(bp, t4lo[:, hh, ncol],
                                             w4r[:, 0, p, :, :],
                                             start=False, stop=False,
                                             perf_mode=DR)
                            nc.tensor.matmul(bp, t4hi[:, hh, ncol],
                                             w4r[:, 1, p, :, :],
                                             start=False, stop=(p == 1),
                                             perf_mode=DR)
                        cpmul(q, ost[:, q, 448:960], bp, 1.0 / (GS * GW))
                    def ap_block(q, dma_pair):
                        j = 4 * f + q
                        ncol = slice(j * 128, (j + 1) * 128)
                        ap = ops.tile([128, 512], F32, tag="o", name=f"ap{j}")
                        nc.tensor.matmul(ap[:, 192:448], b01[:, ncol],
                                         w3_sb[:, 0, :], start=True, stop=False)
                        nc.tensor.matmul(ap[:, 192:448], b23[:, ncol],
                                         w3_sb[:, 1, :], start=False, stop=True)
                        nc.tensor.matmul(ap[:, 0:192], cdA[:, ncol], wcdA_sb,
                                         start=True, stop=False)
                        nc.tensor.matmul(ap[:, 0:192], cdB[:, ncol], wcdB_sb,
                                         start=False, stop=True)
                        cpmul(q + 1, ost[:, q, 0:448], ap[:, 0:448], 1.0 / GS)
                        if not dma_pair:
                            nc.sync.dma_start(out=out_r[:, j:j + 1, :],
                                              in_=ost[:, q:q + 1, :])
                        elif q % 2 == 1:
                            nc.sync.dma_start(
                                out=out_r[:, j - 1:j + 1, :],
                                in_=ost[:, q - 1:q + 1, :])
                    if interleave:
                        for q in range(4):
                            bp_block(q)
                            ap_block(q, dma_pair=False)
                    else:
                        for q in range(4):
                            bp_block(q)
                        for q in range(4):
                            ap_block(q, dma_pair=(f != NF - 1))

                # software-pipelined by one f-block: ph4 consumes f-1 while
                # ph3 produces f (f0 already ran fused into phase 2)
                for step in range(1, NF + 1):
                    if step < NF:
                        ph3_block(step)
                    ph4_block(step - 1, interleave=(step == NF))
    nc.finalize()
    return nc


def _host_pack(inputs, b, side, qdt):
    if side == 0:
        embs = [inputs['emb1'], inputs['emb2'], inputs['emb3'], inputs['emb4']]
        wq = [inputs[f'wq{i+1}'] for i in range(4)]
        kvsrc = inputs['emb_alld']
    else:
        embs = [inputs['embd1'], inputs['embd2'], inputs['embd3'], inputs['embd4']]
        wq = [inputs[f'wqd{i+1}'] for i in range(4)]
        kvsrc = inputs['emb_all']
    scale = np.float32(1.0 / np.sqrt(np.float32(KV)))
    eq = np.empty((N, 960), np.float32)
    kvf = np.empty((N, 960), np.float32)
    for h in range(H):
        for i in range(4):
            cq = CQ[i]
            blkq = np.asarray(embs[i][b][:, h * cq:(h + 1) * cq], np.float32)
            # fold wq (and 1/sqrt(KV)) into the q-side pack: Q = emb @ wq^T
            eq[:, h * 240 + QOFF[i]: h * 240 + QOFF[i] + cq] = \
                (blkq @ np.asarray(wq[i][h], np.float32).T) * scale
            kvf[:, h * 240 + KOFF[i]: h * 240 + KOFF[i] + cq] = \
                kvsrc[b][:, RAW[i] + h * cq: RAW[i] + (h + 1) * cq]

    # kvT e4m3 hi/lo pack: [128, f hl h u c], u1 partitions 112:128 zero
    kvt = np.zeros((128, NF, 2, H, 2, 512), np.float32)
    for h in range(H):
        kT = kvf[:, h * 240:(h + 1) * 240].T  # (240, N)
        for u, (r0, r1) in enumerate(((0, 128), (128, 240))):
            blk = kT[r0:r1].reshape(r1 - r0, NF, 512)
            hi = blk.astype(E4NP).astype(np.float32)
            lo = (blk - hi).astype(E4NP).astype(np.float32)
            kvt[0:r1 - r0, :, 0, h, u, :] = hi.transpose(0, 1, 2)
            kvt[0:r1 - r0, :, 1, h, u, :] = lo
    kvthl = np.ascontiguousarray(kvt.reshape(128, -1)).astype(E4NP)

    if qdt is E3:
        # per-head power-of-2 scaling into the e3m4 sweet spot (absmax ~12);
        # inorm makes S invariant to any per-head constant factor
        for h in range(H):
            cs = slice(h * 240, (h + 1) * 240)
            eq[:, cs] *= 2.0 ** np.floor(np.log2(12.0 / np.abs(eq[:, cs]).max()))
            kvf[:, cs] *= 2.0 ** np.floor(np.log2(12.0 / np.abs(kvf[:, cs]).max()))
        qnp = E3NP
    else:
        qnp = BF
    eqp = np.ascontiguousarray(
        eq.reshape(NT, 128, 960).transpose(1, 0, 2)).reshape(128, NT * 960)
    kvp = np.ascontiguousarray(
        kvf.reshape(NT, 128, 960).transpose(1, 0, 2)).reshape(128, NT * 960)
    return eqp.astype(qnp), kvp.astype(qnp), kvthl


def _host_weights(inputs, side):
    if side == 0:
        wk, wv = inputs['wkd'], inputs['wvd']
        wout = [inputs[f'wout{i+1}'] for i in range(4)]
    else:
        wk, wv = inputs['wk'], inputs['wv']
        wout = [inputs[f'woutd{i+1}'] for i in range(4)]
    wkT = np.asarray(wk).T.astype(np.float32)
    wkt = np.zeros((128, 2, 241), np.float32)
    wkt[:, 0, 0:240] = wkT[0:128, :]
    wkt[0:112, 1, 0:240] = wkT[128:240, :]
    wkt[:, 0, 240] = wkT[0:128, :].sum(axis=1)
    wkt[0:112, 1, 240] = wkT[128:240, :].sum(axis=1)
    wvf = np.asarray(wv).astype(np.float32) * np.float32(GS)
    wvs = np.zeros((128, 2, 240), np.float32)
    wvs[:, 0, :] = wvf[0:128, :]
    wvs[0:112, 1, :] = wvf[128:240, :]
    w3t = np.asarray(wout[2]).T.astype(np.float32)
    w3pk = np.stack([w3t[0:128, :], w3t[128:256, :]], axis=1)
    w2pk = np.asarray(wout[1]).T.astype(np.float32)
    w1t = np.asarray(wout[0]).T.astype(np.float32)
    wcd = np.zeros((2, 128, 192), np.float32)
    for h in range(H):
        t, o = divmod(h, 2)
        wcd[t, o * 64 + 0:o * 64 + 32, 64:192] = w2pk[h * 32:(h + 1) * 32, :]
        wcd[t, o * 64 + 32:o * 64 + 48, 0:64] = w1t[h * 16:(h + 1) * 16, :]
    wallh = np.zeros((128, W_END), np.float32)
    wallh[:, 0:128] = np.eye(128, dtype=np.float32)
    wallh[:, W_WKT:W_WVS] = wkt.reshape(128, 482)
    wallh[:, W_WVS:W_W3] = wvs.reshape(128, 480)
    wallh[:, W_W3:W_CDA] = w3pk.reshape(128, 512)
    wallh[:, W_CDA:W_CDB] = wcd[0]
    wallh[:, W_CDB:W_ZZ] = wcd[1]

    # w4 * GW as e4m3 hi/lo: [128, hl pair kt c]
    w4t = np.asarray(wout[3]).T.astype(np.float32) * np.float32(GW)
    w4pk = np.stack([w4t[h * 128:(h + 1) * 128, :] for h in range(H)], axis=1)
    w4a = np.zeros((128, 2, 2, 2, 512), np.float32)
    for p in range(2):
        for k in range(2):
            blk = w4pk[:, 2 * p + k, :]
            hi = blk.astype(E4NP).astype(np.float32)
            w4a[:, 0, p, k, :] = hi
            w4a[:, 1, p, k, :] = (blk - hi).astype(E4NP).astype(np.float32)
    w4hl = np.ascontiguousarray(w4a.reshape(128, 4096)).astype(E4NP)
    return dict(wall=wallh.astype(BF), w4hl=w4hl)


def _host_consts():
    # indb pre-scaled by 1/(nblk*240) so the indicator matmul yields means
    indb = np.zeros((128, 8), np.float32)
    indb[:, 0] = 1.0 / (128 * 240)
    indb[0:64, 5] = 1.0 / (64 * 240)
    indb[64:96, 6] = 1.0 / (32 * 240)
    indb[96:112, 7] = 1.0 / (16 * 240)
    indc = np.zeros((4, 240), np.float32)
    indc[0, 0:128] = 1.0
    indc[1, 128:192] = 1.0
    indc[2, 192:224] = 1.0
    indc[3, 224:240] = 1.0
    return dict(indb=indb, indc=indc)


def _in_map(inputs, b, side, wside, consts):
    eqp, kvp, kvthl = _host_pack(inputs, b, side, QDT)
    return dict(eqp=eqp, kvp=kvp, kvthl=kvthl, **wside[side], **consts)


def _unpack_out(raw):
    o = np.asarray(raw).reshape(128, NT, 960).transpose(1, 0, 2)
    return np.ascontiguousarray(o).reshape(N, 960).astype(np.float32)


def kernel(**inputs):
    inputs = {k: np.asarray(v, dtype=np.float32) for k, v in inputs.items()}
    key = ("nc", str(QDT))
    if key not in _CACHE:
        _CACHE[key] = _build_bass(QDT)
        _CACHE["nc"] = _CACHE[key]
    nc = _CACHE[key]
    consts = _host_consts()
    wside = [_host_weights(inputs, 0), _host_weights(inputs, 1)]
    in_maps = [_in_map(inputs, core // 2, core % 2, wside, consts)
               for core in range(8)]
    res = run_bass_kernel_spmd(nc, in_maps, list(range(8)))
    out = np.empty((B, N, 2 * KV), np.float32)
    for core in range(8):
        b, side = core // 2, core % 2
        out[b, :, side * 960:(side + 1) * 960] = _unpack_out(res.results[core]["outp"])
    return out


# revision 8
# speedup vs baseline: 1.2089x; 1.0582x over previous
"""Trainium2 Bass kernel for nn_Attention_org_cross (cross-modal channel attention).

Sharding: 8 cores = 4 batches x 2 modality directions (pure data parallel).
Core (b, side=0): optical queries attend to DSM K/V -> out[b,:,0:960]; side=1 reverse.

v2 data path:
  ph1: eq/kv streamed as fp8-e3m4 (host power-of-2 scaled; inorm makes S
       scale-invariant) -> Ut[h] in PSUM.  No PE transposes: kvT comes
       pre-transposed from DRAM as e4m3 hi/lo pairs.
  ph2: S = Ut^T wkt (col 240 = row-sums); per-scale var -> r; exp(r*s);
       pr *= 1/esum (softmax fold); PE-transpose; P2 = (wv*g) @ pt;
       evac P2 as e4m3 hi/lo.
  ph3: ctxT = P2 @ kvT via fp8 DoubleRow hi/lo (3 products, lo*lo dropped);
       s4-part evac as e4m3 hi/lo (t4), s3/s2/s1 bf16 (tc1 -> b/cd repack).
  ph4: s4 out = t4 @ (w4*gw) DoubleRow hi/lo; w3/w2/w1 bf16; ost evac
       applies 1/(g*gw) resp 1/g.
"""
import sys

sys.path.insert(0, "/opt/trn_rl_repo")

import numpy as np
import ml_dtypes

import concourse.bacc as bacc
import concourse.mybir as mybir
import concourse.tile as tile
from concourse.bass_utils import run_bass_kernel_spmd

F32 = mybir.dt.float32
BF16 = mybir.dt.bfloat16
E4 = mybir.dt.float8e4
E3 = mybir.dt.float8e3
BF = ml_dtypes.bfloat16
E4NP = ml_dtypes.float8_e4m3
E3NP = ml_dtypes.float8_e3m4
MULT = mybir.AluOpType.mult
SUB = mybir.AluOpType.subtract
AExp = mybir.ActivationFunctionType.Exp
ASqrt = mybir.ActivationFunctionType.Sqrt
DR = mybir.MatmulPerfMode.DoubleRow

B, N, H, KV = 4, 4096, 4, 960
CQ = (16, 32, 64, 128)
RAW = (0, 64, 192, 448)
QOFF = (224, 192, 128, 0)        # scale i -> within-head q offset ([s4 s3 s2 s1])
KOFF = (0, 16, 48, 112)          # kv-side within-head offsets ([s1 s2 s3 s4])
EPS = 1e-5
NT = 32
NGRP = 2
NF = 8

GS = 128.0                       # wv scale (ctx = kv@G^T carries GS)
GW = 128.0                       # w4 scale
QDT = E3                         # ph1 stream dtype: E3 (fast) or BF16 (safe)

# wall offsets: idt | wkt | wvs*g | w3 | wcdA | wcdB | zz
W_IDT, W_WKT, W_WVS, W_W3, W_CDA, W_CDB, W_ZZ, W_END = \
    0, 128, 610, 1090, 1602, 1794, 1986, 2498

_CACHE = {}


def _build_bass(qdt):
    nc = bacc.Bacc(trn_type="TRN2", target_bir_lowering=False, debug=False)

    eqp = nc.declare_dram_parameter("eqp", [128, NT * 960], qdt, isOutput=False)
    kvp = nc.declare_dram_parameter("kvp", [128, NT * 960], qdt, isOutput=False)
    # kvthl: [128, f(8) hl(2) h(4) u(2) c(512)] e4m3, u1 partitions 112:128 zero
    kvthl = nc.declare_dram_parameter("kvthl", [128, 8 * 2 * 4096], E4,
                                      isOutput=False)
    wall = nc.declare_dram_parameter("wall", [128, W_END], BF16, isOutput=False)
    w4hl = nc.declare_dram_parameter("w4hl", [128, 4096], E4, isOutput=False)
    indb = nc.declare_dram_parameter("indb", [128, 8], F32, isOutput=False)
    indc = nc.declare_dram_parameter("indc", [4, 240], F32, isOutput=False)
    outp = nc.declare_dram_parameter("outp", [128, NT * 960], BF16, isOutput=True)

    eq_r = eqp.rearrange("p (t c) -> p t c", t=NT)
    kv_r = kvp.rearrange("p (t c) -> p t c", t=NT)
    kvt_r = kvthl.rearrange("p (f l c) -> p f l c", f=NF, l=2)
    out_r = outp.rearrange("p (t c) -> p t c", t=NT)

    def cp(e, out, in_):
        if e % 2 == 0:
            nc.vector.tensor_copy(out, in_)
        else:
            nc.scalar.copy(out, in_)

    def cpmul(e, out, in_, s):
        if e % 2 == 0:
            nc.vector.tensor_scalar(out=out, in0=in_, scalar1=s, scalar2=None,
                                    op0=MULT)
        else:
            nc.scalar.mul(out, in_, s)

    with tile.TileContext(nc) as tc:
        from contextlib import ExitStack
        with ExitStack() as outer:
            wts = outer.enter_context(tc.tile_pool(name="wts", bufs=1))
            p2sb = outer.enter_context(tc.tile_pool(name="p2sb", bufs=1))
            gsbp = outer.enter_context(tc.tile_pool(name="gsbp", bufs=1))
            esp = outer.enter_context(tc.tile_pool(name="esp", bufs=1))
            gstack = ExitStack()
            gbp = gstack.enter_context(tc.tile_pool(name="gbp", bufs=1, space="PSUM"))

            wall_sb = wts.tile([128, W_END], BF16, tag="wall")
            nc.sync.dma_start(out=wall_sb[:, 0:128], in_=wall[:, 0:128])
            nc.vector.memset(wall_sb[0:1, W_ZZ:W_END], 0.0)
            idt = wall_sb[:, W_IDT:W_IDT + 128]
            wkt_sb = wall_sb[:, W_WKT:W_WVS].rearrange("p (u c) -> p u c", u=2)
            wvs_sb = wall_sb[:, W_WVS:W_W3].rearrange("p (u c) -> p u c", u=2)
            w3_sb = wall_sb[:, W_W3:W_CDA].rearrange("p (u c) -> p u c", u=2)
            wcdA_sb = wall_sb[:, W_CDA:W_CDB]
            wcdB_sb = wall_sb[:, W_CDB:W_ZZ]
            zzt = wall_sb[0:1, W_ZZ:W_END]
            w4sb = wts.tile([128, 4096], E4, tag="w4sb")
            w4r = w4sb.rearrange("p (l q k c) -> p l q k c", l=2, q=2, k=2)
            indb_sb = wts.tile([128, 8], F32, tag="indb")
            indc_sb = wts.tile([4, 240], F32, tag="indc")
            eps_t = wts.tile([4, 1], F32, tag="eps")
            nc.scalar.dma_start(out=indb_sb, in_=indb[:, :])
            nc.scalar.dma_start(out=indc_sb, in_=indc[:, :])
            nc.vector.memset(eps_t, EPS)
            # preload the sqrt act table off the critical path
            sqd = wts.tile([4, 1], F32, tag="sqd")
            nc.scalar.activation(out=sqd, in_=eps_t, func=ASqrt, bias=eps_t)
            nc.scalar.activation(out=sqd, in_=eps_t, func=AExp)

            p2hi = [p2sb.tile([128, 2, 240], E4, tag=f"p2h{h}", name=f"p2h{h}")
                    for h in range(H)]
            p2lo = [p2sb.tile([128, 2, 240], E4, tag=f"p2l{h}", name=f"p2l{h}")
                    for h in range(H)]
            for h in range(H):
                # u1 garbage partitions must be finite: lhsT rows 112:128 of
                # ktile 1 multiply kvT's zero rows (host-padded).  Engine
                # accesses must start at a x32 partition, so clear 96:128
                # before the evac rewrites 96:112.
                nc.vector.memset(p2hi[h][96:128, 1, :], 0.0)
                nc.vector.memset(p2lo[h][96:128, 1, :], 0.0)
            gsb = [gsbp.tile([128, 2, 240], BF16, tag=f"gsb{h}", name=f"gsb{h}")
                   for h in range(H)]
            esr = [esp.tile([128, 2], F32, tag=f"esr{h}", name=f"esr{h}")
                   for h in range(H)]
            gb = [gbp.tile([128, 480], F32, tag=f"g{h}", name=f"g{h}")
                  for h in range(H)]

            stream = outer.enter_context(tc.tile_pool(name="stream", bufs=4))
            sm = outer.enter_context(tc.tile_pool(name="sm", bufs=1))
            ops = None  # opened just before phase 2

            # ---- phase 1: Ut accumulation (lhsT=kv) ----
            if True:
                for h in range(H):
                    nc.tensor.matmul(gb[h], zzt[0:1, 0:128], zzt[0:1, 0:480],
                                     start=True, stop=False)
                for g in range(NT // NGRP):
                    eq_t = stream.tile([128, NGRP, 960], qdt, tag="eq")
                    kv_t = stream.tile([128, NGRP, 960], qdt, tag="kv")
                    j0 = g * NGRP
                    nc.sync.dma_start(out=kv_t, in_=kv_r[:, j0:j0 + NGRP, :])
                    nc.sync.dma_start(out=eq_t, in_=eq_r[:, j0:j0 + NGRP, :])
                    if g == 8:
                        nc.scalar.dma_start(out=wall_sb[:, 128:W_ZZ],
                                            in_=wall[:, 128:W_ZZ])
                    if g == 10:
                        nc.scalar.dma_start(out=w4sb, in_=w4hl[:, :])
                    for h in range(H):
                        hq = hk = h * 240
                        for jj in range(NGRP):
                            nc.tensor.matmul(
                                gb[h][:, 0:240],
                                kv_t[:, jj, hk:hk + 128],
                                eq_t[:, jj, hq:hq + 240],
                                start=False, stop=False)
                            nc.tensor.matmul(
                                gb[h][0:112, 240:480],
                                kv_t[:, jj, hk + 128:hk + 240],
                                eq_t[:, jj, hq:hq + 240],
                                start=False, stop=False)
                for h in range(H):
                    nc.tensor.matmul(gb[h], zzt[0:1, 0:128], zzt[0:1, 0:480],
                                     start=False, stop=True)

            # ---- Ut evac, free banks ----
            for h in range(H):
                cp(h, gsb[h][:, 0, :], gb[h][:, 0:240])
                cp(h + 1, gsb[h][0:112, 1, :], gb[h][0:112, 240:480])
            gstack.close()

            # ---- phase 3/4 static tiles (allocated early so ph3(f0) can
            # run fused into phase 2) ----
            ctg = outer.enter_context(tc.tile_pool(name="ctg", bufs=1))
            ostp = outer.enter_context(tc.tile_pool(name="ostp", bufs=3))
            kvs = outer.enter_context(tc.tile_pool(name="kvs", bufs=3))
            t4hi = ctg.tile([128, 4, N], E4, tag="t4hi")
            t4lo = ctg.tile([128, 4, N], E4, tag="t4lo")
            tc1 = [ctg.tile([112, N], BF16, tag=f"tc1{h}", name=f"tc1{h}")
                   for h in range(H)]
            b01 = ctg.tile([128, N], BF16, tag="b01")
            b23 = ctg.tile([128, N], BF16, tag="b23")
            cdA = ctg.tile([128, N], BF16, tag="cdA")
            cdB = ctg.tile([128, N], BF16, tag="cdB")
            nc.vector.memset(cdA, 0.0)
            nc.gpsimd.memset(cdB, 0.0)

            def kvt_load(f, gate):
                kh = kvs.tile([128, 4096], E4, tag="kh", name=f"kh{f}")
                kl = kvs.tile([128, 4096], E4, tag="kl", name=f"kl{f}")
                if gate:
                    # order the DMA behind phase 1 (gsb is written at its
                    # end) so the transfer cannot steal ph1 stream bandwidth
                    nc.scalar.copy(kh[:, 0:1], gsb[0][:, 0, 0:1])
                    nc.scalar.copy(kl[:, 0:1], gsb[1][:, 0, 0:1])
                nc.sync.dma_start(out=kh, in_=kvt_r[:, f, 0, :])
                nc.sync.dma_start(out=kl, in_=kvt_r[:, f, 1, :])
                return (kh.rearrange("p (h u c) -> p h u c", h=4, u=2),
                        kl.rearrange("p (h u c) -> p h u c", h=4, u=2))

            def ph3_head(f, h, khr, klr, c0, c1):
                fc = slice(f * 512, (f + 1) * 512)
                a0h = p2hi[h][:, :, 0:128]
                a0l = p2lo[h][:, :, 0:128]
                a1h = p2hi[h][:, :, 128:240]
                a1l = p2lo[h][:, :, 128:240]
                bh, bl = khr[:, h, :, :], klr[:, h, :, :]
                nc.tensor.matmul(c0, a0h, bh, start=True, stop=False,
                                 perf_mode=DR)
                nc.tensor.matmul(c0, a0l, bh, start=False, stop=False,
                                 perf_mode=DR)
                nc.tensor.matmul(c0, a0h, bl, start=False, stop=True,
                                 perf_mode=DR)
                nc.tensor.matmul(c1, a1h, bh, start=True, stop=False,
                                 perf_mode=DR)
                nc.tensor.matmul(c1, a1l, bh, start=False, stop=False,
                                 perf_mode=DR)
                nc.tensor.matmul(c1, a1h, bl, start=False, stop=True,
                                 perf_mode=DR)
                o = (h % 2) * 64
                bt = b01 if h < 2 else b23
                cd = cdA if h < 2 else cdB
                nc.scalar.copy(t4hi[:, h, fc], c0)
                nc.vector.tensor_tensor(out=t4lo[:, h, fc], in0=c0,
                                        in1=t4hi[:, h, fc], op=SUB)
                cp(h + f, tc1[h][:, fc], c1)
                nc.gpsimd.tensor_copy(bt[o:o + 64, fc], tc1[h][0:64, fc])
                nc.gpsimd.tensor_copy(cd[o:o + 48, fc], tc1[h][64:112, fc])

            # ---- phase 2: step-interleaved, with ph3(f0) fused in ----
            khr0, klr0 = kvt_load(0, gate=False)
            ops = outer.enter_context(tc.tile_pool(name="ops", bufs=3, space="PSUM"))
            with tc.tile_pool(name="phw", bufs=4, space="PSUM") as phw, \
                 tc.tile_pool(name="phb", bufs=1, space="PSUM") as phb:
                scp, st0, st1, tiny, var, rall = {}, {}, {}, {}, {}, {}
                rv, pr0, pr1, sq0, sq1 = {}, {}, {}, {}, {}
                for h in range(H):
                    scp[h] = phw.tile([128, 2, 246], F32, tag="w", name=f"scp{h}")
                    tiny[h] = scp[h][:, 0, 242:246]
                    nc.tensor.matmul(scp[h][:, 0, 0:241], gsb[h][:, 0, 0:128],
                                     wkt_sb[:, 0, :], start=True, stop=False)
                    nc.tensor.matmul(scp[h][:, 0, 0:241], gsb[h][0:112, 1, 0:128],
                                     wkt_sb[0:112, 1, :], start=False, stop=True)
                    nc.tensor.matmul(scp[h][0:112, 1, 0:241], gsb[h][:, 0, 128:240],
                                     wkt_sb[:, 0, :], start=True, stop=False)
                    nc.tensor.matmul(scp[h][0:112, 1, 0:241],
                                     gsb[h][0:112, 1, 128:240],
                                     wkt_sb[0:112, 1, :], start=False, stop=True)
                for h in range(H):
                    st0[h] = sm.tile([128, 2], F32, tag=f"st0{h}", name=f"st0{h}")
                    st1[h] = sm.tile([112, 2], F32, tag=f"st1{h}", name=f"st1{h}")
                    nc.vector.tensor_copy(st0[h][:, 0:1], scp[h][:, 0, 240:241])
                    nc.vector.tensor_copy(st1[h][:, 0:1], scp[h][0:112, 1, 240:241])
                    sq0[h] = sm.tile([128, 240], BF16, tag="sqs", bufs=2,
                                     name=f"sq0{h}")
                    sq1[h] = sm.tile([112, 240], BF16, tag="sqs2", bufs=2,
                                     name=f"sq1{h}")
                    nc.scalar.activation(out=sq0[h], in_=scp[h][:, 0, 0:240],
                                         func=mybir.ActivationFunctionType.Square,
                                         accum_out=st0[h][:, 1:2])
                    nc.scalar.activation(out=sq1[h], in_=scp[h][0:112, 1, 0:240],
                                         func=mybir.ActivationFunctionType.Square,
                                         accum_out=st1[h][:, 1:2])
                for h in range(H):
                    nc.tensor.matmul(tiny[h][0:4, 2:4], indb_sb[:, 0:4], st0[h],
                                     start=True, stop=False)
                    nc.tensor.matmul(tiny[h][0:4, 2:4], indb_sb[0:112, 4:8], st1[h],
                                     start=False, stop=True)
                for h in range(H):
                    # indb is pre-scaled by 1/(nblk*240): tiny holds (mean, meansq)
                    var[h] = sm.tile([4, 1], F32, tag=f"var{h}", name=f"var{h}")
                    nc.scalar.activation(out=var[h], in_=tiny[h][0:4, 2:3],
                                         func=mybir.ActivationFunctionType.Square)
                    nc.vector.tensor_tensor(out=var[h], in0=tiny[h][0:4, 3:4],
                                            in1=var[h], op=SUB)
                for h in range(H):
                    rall[h] = sm.tile([4, 1], F32, tag=f"rall{h}", name=f"rall{h}")
                    nc.scalar.activation(out=rall[h], in_=var[h], func=ASqrt,
                                         bias=eps_t)
                    nc.vector.reciprocal(out=rall[h], in_=rall[h])
                for h in range(H):
                    nc.tensor.matmul(tiny[h][:, 0:1], indc_sb[:, 0:128], rall[h],
                                     start=True, stop=True)
                    nc.tensor.matmul(tiny[h][0:112, 1:2], indc_sb[:, 128:240],
                                     rall[h], start=True, stop=True)
                    rv[h] = sm.tile([128, 2], F32, tag=f"rv{h}", name=f"rv{h}")
                    nc.vector.tensor_copy(rv[h][:, 0:1], tiny[h][:, 0:1])
                    nc.vector.tensor_copy(rv[h][0:112, 1:2], tiny[h][0:112, 1:2])
                for h in range(H):
                    # inorm guarantees ~unit-variance scores: shift-free softmax
                    pr0[h] = sm.tile([128, 240], BF16, tag=f"pr0{h}", name=f"pr0{h}")
                    pr1[h] = sm.tile([112, 240], BF16, tag=f"pr1{h}", name=f"pr1{h}")
                    nc.scalar.activation(out=pr0[h], in_=scp[h][:, 0, 0:240],
                                         func=AExp, scale=rv[h][:, 0:1],
                                         accum_out=esr[h][:, 0:1])
                    nc.scalar.activation(out=pr1[h], in_=scp[h][0:112, 1, 0:240],
                                         func=AExp, scale=rv[h][0:112, 1:2],
                                         accum_out=esr[h][0:112, 1:2])
                    nc.vector.reciprocal(out=esr[h][:, 0:1], in_=esr[h][:, 0:1])
                    nc.vector.reciprocal(out=esr[h][0:112, 1:2],
                                         in_=esr[h][0:112, 1:2])
                for h in range(H):
                    # fold softmax 1/esum into P before the transpose
                    nc.vector.tensor_scalar(out=pr0[h], in0=pr0[h],
                                            scalar1=esr[h][:, 0:1], scalar2=None,
                                            op0=MULT)
                    nc.gpsimd.tensor_scalar(out=pr1[h], in0=pr1[h],
                                            scalar1=esr[h][0:112, 1:2],
                                            scalar2=None, op0=MULT)
                pt = {}
                for h in range(H):
                    ptp = phb.tile([128, 2, 240], BF16, tag="b", name=f"ptp{h}")
                    nc.tensor.transpose(ptp[:, 0, 0:128], pr0[h][:, 0:128], idt)
                    nc.tensor.transpose(ptp[0:112, 1, 0:128], pr0[h][:, 128:240],
                                        idt)
                    nc.tensor.transpose(ptp[:, 0, 128:240], pr1[h][:, 0:128],
                                        idt[0:112, 0:112])
                    nc.tensor.transpose(ptp[0:112, 1, 128:240],
                                        pr1[h][:, 128:240], idt[0:112, 0:112])
                    pt[h] = sm.tile([128, 2, 240], BF16, tag=f"pt{h}",
                                    name=f"pt{h}")
                    cp(0, pt[h][:, 0, :], ptp[:, 0, :])
                    cp(1, pt[h][0:112, 1, :], ptp[0:112, 1, :])
                    p2p = phw.tile([128, 2, 246], F32, tag="w", name=f"p2p{h}")
                    nc.tensor.matmul(p2p[:, 0, 0:240], wvs_sb[:, 0, 0:128],
                                     pt[h][:, 0, :], start=True, stop=False)
                    nc.tensor.matmul(p2p[:, 0, 0:240], wvs_sb[0:112, 1, 0:128],
                                     pt[h][0:112, 1, :], start=False, stop=True)
                    nc.tensor.matmul(p2p[0:112, 1, 0:240], wvs_sb[:, 0, 128:240],
                                     pt[h][:, 0, :], start=True, stop=False)
                    nc.tensor.matmul(p2p[0:112, 1, 0:240], wvs_sb[0:112, 1, 128:240],
                                     pt[h][0:112, 1, :], start=False, stop=True)
                    # e4m3 hi/lo evac of G^T (scaled by GS via wvs)
                    nc.scalar.copy(p2hi[h][:, 0, :], p2p[:, 0, 0:240])
                    nc.vector.tensor_tensor(out=p2lo[h][:, 0, :],
                                            in0=p2p[:, 0, 0:240],
                                            in1=p2hi[h][:, 0, :], op=SUB)
                    nc.scalar.copy(p2hi[h][0:112, 1, :], p2p[0:112, 1, 0:240])
                    nc.vector.tensor_tensor(out=p2lo[h][0:112, 1, :],
                                            in0=p2p[0:112, 1, 0:240],
                                            in1=p2hi[h][0:112, 1, :], op=SUB)
                    # ph3(f0) for the previous head fills PE while this
                    # head's G evac completes
                    if h > 0:
                        c0 = ops.tile([128, 512], F32, tag="o", name=f"fc0{h}")
                        c1 = ops.tile([112, 512], F32, tag="o", name=f"fc1{h}")
                        ph3_head(0, h - 1, khr0, klr0, c0, c1)
                c0 = ops.tile([128, 512], F32, tag="o", name="fc0z")
                c1 = ops.tile([112, 512], F32, tag="o", name="fc1z")
                ph3_head(0, 3, khr0, klr0, c0, c1)

            # ---- phases 3+4 fully fused per f-block ----
            with tc.tile_pool(name="cps", bufs=3, space="PSUM") as cps, \
                 tc.tile_pool(name="cps1", bufs=2, space="PSUM") as cps1:
                def ph3_block(f):
                    khr, klr = kvt_load(f, gate=(f in (1, 2)))
                    for h in range(H):
                        c0 = cps.tile([128, 512], F32, tag="c0")
                        c1 = cps1.tile([112, 512], F32, tag="c1")
                        ph3_head(f, h, khr, klr, c0, c1)

                def ph4_block(f, interleave=False):
                    ost = ostp.tile([128, 4, 960], BF16, tag="ost")
                    def bp_block(q):
                        j = 4 * f + q
                        ncol = slice(j * 128, (j + 1) * 128)
                        bp = ops.tile([128, 512], F32, tag="o", name=f"bp{j}")
                        for p in range(2):
                            hh = slice(2 * p, 2 * p + 2)
                            nc.tensor.matmul(bp, t4hi[:, hh, ncol],
                                             w4r[:, 0, p, :, :],
                                             start=(p == 0), stop=False,
                                             perf_mode=DR)
                            nc.tensor.matmul(bp, t4lo[:, hh, ncol],
                                             w4r[:, 0, p, :, :],
                                             start=False, stop=False,
                                             perf_mode=DR)
                            nc.tensor.matmul(bp, t4hi[:, hh, ncol],
                                             w4r[:, 1, p, :, :],
                                             start=False, stop=(p == 1),
                                             perf_mode=DR)
                        cpmul(q, ost[:, q, 448:960], bp, 1.0 / (GS * GW))
                    def ap_block(q, dma_pair):
                        j = 4 * f + q
                        ncol = slice(j * 128, (j + 1) * 128)
                        ap = ops.tile([128, 512], F32, tag="o", name=f"ap{j}")
                        nc.tensor.matmul(ap[:, 192:448], b01[:, ncol],
                                         w3_sb[:, 0, :], start=True, stop=False)
                        nc.tensor.matmul(ap[:, 192:448], b23[:, ncol],
                                         w3_sb[:, 1, :], start=False, stop=True)
                        nc.tensor.matmul(ap[:, 0:192], cdA[:, ncol], wcdA_sb,
                                         start=True, stop=False)
                        nc.tensor.matmul(ap[:, 0:192], cdB[:, ncol], wcdB_sb,
                                         start=False, stop=True)
                        cpmul(q + 1, ost[:, q, 0:448], ap[:, 0:448], 1.0 / GS)
                        if not dma_pair:
                            nc.sync.dma_start(out=out_r[:, j:j + 1, :],
                                              in_=ost[:, q:q + 1, :])
                        elif q % 2 == 1:
                            nc.sync.dma_start(
                                out=out_r[:, j - 1:j + 1, :],
                                in_=ost[:, q - 1:q + 1, :])
                    if interleave:
                        for q in range(4):
                            bp_block(q)
                            ap_block(q, dma_pair=False)
                    else:
                        for q in range(4):
                            bp_block(q)
                        for q in range(4):
                            ap_block(q, dma_pair=(f != NF - 1))

                # software-pipelined by one f-block: ph4 consumes f-1 while
                # ph3 produces f (f0 already ran fused into phase 2)
                for step in range(1, NF + 1):
                    if step < NF:
                        ph3_block(step)
                    ph4_block(step - 1, interleave=(step == NF))
    nc.finalize()
    return nc


def _host_pack(inputs, b, side, qdt):
    if side == 0:
        embs = [inputs['emb1'], inputs['emb2'], inputs['emb3'], inputs['emb4']]
        wq = [inputs[f'wq{i+1}'] for i in range(4)]
        kvsrc = inputs['emb_alld']
    else:
        embs = [inputs['embd1'], inputs['embd2'], inputs['embd3'], inputs['embd4']]
        wq = [inputs[f'wqd{i+1}'] for i in range(4)]
        kvsrc = inputs['emb_all']
    scale = np.float32(1.0 / np.sqrt(np.float32(KV)))
    eq = np.empty((N, 960), np.float32)
    kvf = np.empty((N, 960), np.float32)
    for h in range(H):
        for i in range(4):
            cq = CQ[i]
            blkq = np.asarray(embs[i][b][:, h * cq:(h + 1) * cq], np.float32)
            # fold wq (and 1/sqrt(KV)) into the q-side pack: Q = emb @ wq^T
            eq[:, h * 240 + QOFF[i]: h * 240 + QOFF[i] + cq] = \
                (blkq @ np.asarray(wq[i][h], np.float32).T) * scale
            kvf[:, h * 240 + KOFF[i]: h * 240 + KOFF[i] + cq] = \
                kvsrc[b][:, RAW[i] + h * cq: RAW[i] + (h + 1) * cq]

    # kvT e4m3 hi/lo pack: [128, f hl h u c], u1 partitions 112:128 zero
    kvt = np.zeros((128, NF, 2, H, 2, 512), np.float32)
    for h in range(H):
        kT = kvf[:, h * 240:(h + 1) * 240].T  # (240, N)
        for u, (r0, r1) in enumerate(((0, 128), (128, 240))):
            blk = kT[r0:r1].reshape(r1 - r0, NF, 512)
            hi = blk.astype(E4NP).astype(np.float32)
            lo = (blk - hi).astype(E4NP).astype(np.float32)
            kvt[0:r1 - r0, :, 0, h, u, :] = hi.transpose(0, 1, 2)
            kvt[0:r1 - r0, :, 1, h, u, :] = lo
    kvthl = np.ascontiguousarray(kvt.reshape(128, -1)).astype(E4NP)

    if qdt is E3:
        # per-head power-of-2 scaling into the e3m4 sweet spot (absmax ~12);
        # inorm makes S invariant to any per-head constant factor
        for h in range(H):
            cs = slice(h * 240, (h + 1) * 240)
            eq[:, cs] *= 2.0 ** np.floor(np.log2(12.0 / np.abs(eq[:, cs]).max()))
            kvf[:, cs] *= 2.0 ** np.floor(np.log2(12.0 / np.abs(kvf[:, cs]).max()))
        qnp = E3NP
    else:
        qnp = BF
    eqp = np.ascontiguousarray(
        eq.reshape(NT, 128, 960).transpose(1, 0, 2)).reshape(128, NT * 960)
    kvp = np.ascontiguousarray(
        kvf.reshape(NT, 128, 960).transpose(1, 0, 2)).reshape(128, NT * 960)
    return eqp.astype(qnp), kvp.astype(qnp), kvthl


def _host_weights(inputs, side):
    if side == 0:
        wk, wv = inputs['wkd'], inputs['wvd']
        wout = [inputs[f'wout{i+1}'] for i in range(4)]
    else:
        wk, wv = inputs['wk'], inputs['wv']
        wout = [inputs[f'woutd{i+1}'] for i in range(4)]
    wkT = np.asarray(wk).T.astype(np.float32)
    wkt = np.zeros((128, 2, 241), np.float32)
    wkt[:, 0, 0:240] = wkT[0:128, :]
    wkt[0:112, 1, 0:240] = wkT[128:240, :]
    wkt[:, 0, 240] = wkT[0:128, :].sum(axis=1)
    wkt[0:112, 1, 240] = wkT[128:240, :].sum(axis=1)
    wvf = np.asarray(wv).astype(np.float32) * np.float32(GS)
    wvs = np.zeros((128, 2, 240), np.float32)
    wvs[:, 0, :] = wvf[0:128, :]
    wvs[0:112, 1, :] = wvf[128:240, :]
    w3t = np.asarray(wout[2]).T.astype(np.float32)
    w3pk = np.stack([w3t[0:128, :], w3t[128:256, :]], axis=1)
    w2pk = np.asarray(wout[1]).T.astype(np.float32)
    w1t = np.asarray(wout[0]).T.astype(np.float32)
    wcd = np.zeros((2, 128, 192), np.float32)
    for h in range(H):
        t, o = divmod(h, 2)
        wcd[t, o * 64 + 0:o * 64 + 32, 64:192] = w2pk[h * 32:(h + 1) * 32, :]
        wcd[t, o * 64 + 32:o * 64 + 48, 0:64] = w1t[h * 16:(h + 1) * 16, :]
    wallh = np.zeros((128, W_END), np.float32)
    wallh[:, 0:128] = np.eye(128, dtype=np.float32)
    wallh[:, W_WKT:W_WVS] = wkt.reshape(128, 482)
    wallh[:, W_WVS:W_W3] = wvs.reshape(128, 480)
    wallh[:, W_W3:W_CDA] = w3pk.reshape(128, 512)
    wallh[:, W_CDA:W_CDB] = wcd[0]
    wallh[:, W_CDB:W_ZZ] = wcd[1]

    # w4 * GW as e4m3 hi/lo: [128, hl pair kt c]
    w4t = np.asarray(wout[3]).T.astype(np.float32) * np.float32(GW)
    w4pk = np.stack([w4t[h * 128:(h + 1) * 128, :] for h in range(H)], axis=1)
    w4a = np.zeros((128, 2, 2, 2, 512), np.float32)
    for p in range(2):
        for k in range(2):
            blk = w4pk[:, 2 * p + k, :]
            hi = blk.astype(E4NP).astype(np.float32)
            w4a[:, 0, p, k, :] = hi
            w4a[:, 1, p, k, :] = (blk - hi).astype(E4NP).astype(np.float32)
    w4hl = np.ascontiguousarray(w4a.reshape(128, 4096)).astype(E4NP)
    return dict(wall=wallh.astype(BF), w4hl=w4hl)


def _host_consts():
    # indb pre-scaled by 1/(nblk*240) so the indicator matmul yields means
    indb = np.zeros((128, 8), np.float32)
    indb[:, 0] = 1.0 / (128 * 240)
    indb[0:64, 5] = 1.0 / (64 * 240)
    indb[64:96, 6] = 1.0 / (32 * 240)
    indb[96:112, 7] = 1.0 / (16 * 240)
    indc = np.zeros((4, 240), np.float32)
    indc[0, 0:128] = 1.0
    indc[1, 128:192] = 1.0
    indc[2, 192:224] = 1.0
    indc[3, 224:240] = 1.0
    return dict(indb=indb, indc=indc)


def _in_map(inputs, b, side, wside, consts):
    eqp, kvp, kvthl = _host_pack(inputs, b, side, QDT)
    return dict(eqp=eqp, kvp=kvp, kvthl=kvthl, **wside[side], **consts)


def _unpack_out(raw):
    o = np.asarray(raw).reshape(128, NT, 960).transpose(1, 0, 2)
    return np.ascontiguousarray(o).reshape(N, 960).astype(np.float32)


def kernel(**inputs):
    inputs = {k: np.asarray(v, dtype=np.float32) for k, v in inputs.items()}
    key = ("nc", str(QDT))
    if key not in _CACHE:
        _CACHE[key] = _build_bass(QDT)
        _CACHE["nc"] = _CACHE[key]
    nc = _CACHE[key]
    consts = _host_consts()
    wside = [_host_weights(inputs, 0), _host_weights(inputs, 1)]
    in_maps = [_in_map(inputs, core // 2, core % 2, wside, consts)
               for core in range(8)]
    res = run_bass_kernel_spmd(nc, in_maps, list(range(8)))
    out = np.empty((B, N, 2 * KV), np.float32)
    for core in range(8):
        b, side = core // 2, core % 2
        out[b, :, side * 960:(side + 1) * 960] = _unpack_out(res.results[core]["outp"])
    return out


# revision 10
# speedup vs baseline: 1.2135x; 1.0039x over previous
"""Trainium2 Bass kernel for nn_Attention_org_cross (cross-modal channel attention).

Sharding: 8 cores = 4 batches x 2 modality directions (pure data parallel).
Core (b, side=0): optical queries attend to DSM K/V -> out[b,:,0:960]; side=1 reverse.

v2 data path:
  ph1: eq/kv streamed as fp8-e3m4 (host power-of-2 scaled; inorm makes S
       scale-invariant) -> Ut[h] in PSUM.  No PE transposes: kvT comes
       pre-transposed from DRAM as e4m3 hi/lo pairs.
  ph2: S = Ut^T wkt (col 240 = row-sums); per-scale var -> r; exp(r*s);
       pr *= 1/esum (softmax fold); PE-transpose; P2 = (wv*g) @ pt;
       evac P2 as e4m3 hi/lo.
  ph3: ctxT = P2 @ kvT via fp8 DoubleRow hi/lo (3 products, lo*lo dropped);
       s4-part evac as e4m3 hi/lo (t4), s3/s2/s1 bf16 (tc1 -> b/cd repack).
  ph4: s4 out = t4 @ (w4*gw) DoubleRow hi/lo; w3/w2/w1 bf16; ost evac
       applies 1/(g*gw) resp 1/g.
"""
import sys

sys.path.insert(0, "/opt/trn_rl_repo")

import numpy as np
import ml_dtypes

import concourse.bacc as bacc
import concourse.mybir as mybir
import concourse.tile as tile
from concourse.bass_utils import run_bass_kernel_spmd

F32 = mybir.dt.float32
BF16 = mybir.dt.bfloat16
E4 = mybir.dt.float8e4
E3 = mybir.dt.float8e3
BF = ml_dtypes.bfloat16
E4NP = ml_dtypes.float8_e4m3
E3NP = ml_dtypes.float8_e3m4
MULT = mybir.AluOpType.mult
SUB = mybir.AluOpType.subtract
AExp = mybir.ActivationFunctionType.Exp
ASqrt = mybir.ActivationFunctionType.Sqrt
DR = mybir.MatmulPerfMode.DoubleRow

B, N, H, KV = 4, 4096, 4, 960
CQ = (16, 32, 64, 128)
RAW = (0, 64, 192, 448)
QOFF = (224, 192, 128, 0)        # scale i -> within-head q offset ([s4 s3 s2 s1])
KOFF = (0, 16, 48, 112)          # kv-side within-head offsets ([s1 s2 s3 s4])
EPS = 1e-5
NT = 32
NGRP = 2
NF = 8

GS = 128.0                       # wv scale (ctx = kv@G^T carries GS)
GW = 128.0                       # w4 scale
QDT = E3                         # ph1 stream dtype: E3 (fast) or BF16 (safe)

# wall offsets: idt | wkt | wvs*g | w3 | wcdA | wcdB | zz
W_IDT, W_WKT, W_WVS, W_W3, W_CDA, W_CDB, W_ZZ, W_END = \
    0, 128, 610, 1090, 1602, 1794, 1986, 2498

_CACHE = {}


def _build_bass(qdt):
    nc = bacc.Bacc(trn_type="TRN2", target_bir_lowering=False, debug=False)

    eqp = nc.declare_dram_parameter("eqp", [128, NT * 960], qdt, isOutput=False)
    kvp = nc.declare_dram_parameter("kvp", [128, NT * 960], qdt, isOutput=False)
    # kvthl: [128, f(8) hl(2) h(4) u(2) c(512)] e4m3, u1 partitions 112:128 zero
    kvthl = nc.declare_dram_parameter("kvthl", [128, 8 * 2 * 4096], E4,
                                      isOutput=False)
    wall = nc.declare_dram_parameter("wall", [128, W_END], BF16, isOutput=False)
    w4hl = nc.declare_dram_parameter("w4hl", [128, 4096], E4, isOutput=False)
    indb = nc.declare_dram_parameter("indb", [128, 8], F32, isOutput=False)
    indc = nc.declare_dram_parameter("indc", [4, 240], F32, isOutput=False)
    outp = nc.declare_dram_parameter("outp", [128, NT * 960], BF16, isOutput=True)

    eq_r = eqp.rearrange("p (t c) -> p t c", t=NT)
    kv_r = kvp.rearrange("p (t c) -> p t c", t=NT)
    kvt_r = kvthl.rearrange("p (f l c) -> p f l c", f=NF, l=2)
    out_r = outp.rearrange("p (t c) -> p t c", t=NT)

    def cp(e, out, in_):
        if e % 2 == 0:
            nc.vector.tensor_copy(out, in_)
        else:
            nc.scalar.copy(out, in_)

    def cpmul(e, out, in_, s):
        if e % 2 == 0:
            nc.vector.tensor_scalar(out=out, in0=in_, scalar1=s, scalar2=None,
                                    op0=MULT)
        else:
            nc.scalar.mul(out, in_, s)

    with tile.TileContext(nc) as tc:
        from contextlib import ExitStack
        with ExitStack() as outer:
            wts = outer.enter_context(tc.tile_pool(name="wts", bufs=1))
            p2sb = outer.enter_context(tc.tile_pool(name="p2sb", bufs=1))
            gsbp = outer.enter_context(tc.tile_pool(name="gsbp", bufs=1))
            esp = outer.enter_context(tc.tile_pool(name="esp", bufs=1))
            gstack = ExitStack()
            gbp = gstack.enter_context(tc.tile_pool(name="gbp", bufs=1, space="PSUM"))

            wall_sb = wts.tile([128, W_END], BF16, tag="wall")
            nc.sync.dma_start(out=wall_sb[:, 0:128], in_=wall[:, 0:128])
            nc.vector.memset(wall_sb[0:1, W_ZZ:W_END], 0.0)
            idt = wall_sb[:, W_IDT:W_IDT + 128]
            wkt_sb = wall_sb[:, W_WKT:W_WVS].rearrange("p (u c) -> p u c", u=2)
            wvs_sb = wall_sb[:, W_WVS:W_W3].rearrange("p (u c) -> p u c", u=2)
            w3_sb = wall_sb[:, W_W3:W_CDA].rearrange("p (u c) -> p u c", u=2)
            wcdA_sb = wall_sb[:, W_CDA:W_CDB]
            wcdB_sb = wall_sb[:, W_CDB:W_ZZ]
            zzt = wall_sb[0:1, W_ZZ:W_END]
            w4sb = wts.tile([128, 4096], E4, tag="w4sb")
            w4r = w4sb.rearrange("p (l q k c) -> p l q k c", l=2, q=2, k=2)
            indb_sb = wts.tile([128, 8], F32, tag="indb")
            indc_sb = wts.tile([4, 240], F32, tag="indc")
            eps_t = wts.tile([4, 1], F32, tag="eps")
            nc.scalar.dma_start(out=indb_sb, in_=indb[:, :])
            nc.scalar.dma_start(out=indc_sb, in_=indc[:, :])
            nc.vector.memset(eps_t, EPS)
            # preload the sqrt act table off the critical path
            sqd = wts.tile([4, 1], F32, tag="sqd")
            nc.scalar.activation(out=sqd, in_=eps_t, func=ASqrt, bias=eps_t)
            nc.scalar.activation(out=sqd, in_=eps_t, func=AExp)

            p2hi = [p2sb.tile([128, 2, 240], E4, tag=f"p2h{h}", name=f"p2h{h}")
                    for h in range(H)]
            p2lo = [p2sb.tile([128, 2, 240], E4, tag=f"p2l{h}", name=f"p2l{h}")
                    for h in range(H)]
            for h in range(H):
                # u1 garbage partitions must be finite: lhsT rows 112:128 of
                # ktile 1 multiply kvT's zero rows (host-padded).  Engine
                # accesses must start at a x32 partition, so clear 96:128
                # before the evac rewrites 96:112.
                nc.vector.memset(p2hi[h][96:128, 1, :], 0.0)
                nc.vector.memset(p2lo[h][96:128, 1, :], 0.0)
            gsb = [gsbp.tile([128, 2, 240], BF16, tag=f"gsb{h}", name=f"gsb{h}")
                   for h in range(H)]
            esr = [esp.tile([128, 2], F32, tag=f"esr{h}", name=f"esr{h}")
                   for h in range(H)]
            gb = [gbp.tile([128, 480], F32, tag=f"g{h}", name=f"g{h}")
                  for h in range(H)]

            stream = outer.enter_context(tc.tile_pool(name="stream", bufs=4))
            sm = outer.enter_context(tc.tile_pool(name="sm", bufs=1))
            ops = None  # opened just before phase 2

            # ---- phase 1: Ut accumulation (lhsT=kv) ----
            if True:
                for h in range(H):
                    nc.tensor.matmul(gb[h], zzt[0:1, 0:128], zzt[0:1, 0:480],
                                     start=True, stop=False)
                for g in range(NT // NGRP):
                    eq_t = stream.tile([128, NGRP, 960], qdt, tag="eq")
                    kv_t = stream.tile([128, NGRP, 960], qdt, tag="kv")
                    j0 = g * NGRP
                    nc.sync.dma_start(out=kv_t, in_=kv_r[:, j0:j0 + NGRP, :])
                    nc.sync.dma_start(out=eq_t, in_=eq_r[:, j0:j0 + NGRP, :])
                    if g == 8:
                        nc.scalar.dma_start(out=wall_sb[:, 128:W_ZZ],
                                            in_=wall[:, 128:W_ZZ])
                    if g == 10:
                        nc.scalar.dma_start(out=w4sb, in_=w4hl[:, :])
                    for h in range(H):
                        hq = hk = h * 240
                        for jj in range(NGRP):
                            nc.tensor.matmul(
                                gb[h][:, 0:240],
                                kv_t[:, jj, hk:hk + 128],
                                eq_t[:, jj, hq:hq + 240],
                                start=False, stop=False)
                            nc.tensor.matmul(
                                gb[h][0:112, 240:480],
                                kv_t[:, jj, hk + 128:hk + 240],
                                eq_t[:, jj, hq:hq + 240],
                                start=False, stop=False)
                for h in range(H):
                    nc.tensor.matmul(gb[h], zzt[0:1, 0:128], zzt[0:1, 0:480],
                                     start=False, stop=True)

            # ---- Ut evac, free banks ----
            for h in range(H):
                cp(h, gsb[h][:, 0, :], gb[h][:, 0:240])
                cp(h + 1, gsb[h][0:112, 1, :], gb[h][0:112, 240:480])
            gstack.close()

            # ---- phase 3/4 static tiles (allocated early so ph3(f0) can
            # run fused into phase 2) ----
            ctg = outer.enter_context(tc.tile_pool(name="ctg", bufs=1))
            ostp = outer.enter_context(tc.tile_pool(name="ostp", bufs=3))
            kvs = outer.enter_context(tc.tile_pool(name="kvs", bufs=3))
            t4hi = ctg.tile([128, 4, N], E4, tag="t4hi")
            t4lo = ctg.tile([128, 4, N], E4, tag="t4lo")
            tc1 = [ctg.tile([112, N], BF16, tag=f"tc1{h}", name=f"tc1{h}")
                   for h in range(H)]
            b01 = ctg.tile([128, N], BF16, tag="b01")
            b23 = ctg.tile([128, N], BF16, tag="b23")
            cdA = ctg.tile([128, N], BF16, tag="cdA")
            cdB = ctg.tile([128, N], BF16, tag="cdB")
            nc.vector.memset(cdA, 0.0)
            nc.gpsimd.memset(cdB, 0.0)

            def kvt_load(f, gate):
                kh = kvs.tile([128, 4096], E4, tag="kh", name=f"kh{f}")
                kl = kvs.tile([128, 4096], E4, tag="kl", name=f"kl{f}")
                if gate:
                    # order the DMA behind phase 1 (gsb is written at its
                    # end) so the transfer cannot steal ph1 stream bandwidth
                    nc.vector.tensor_copy(kh[:, 0:1], gsb[0][:, 0, 0:1])
                    nc.vector.tensor_copy(kl[:, 0:1], gsb[1][:, 0, 0:1])
                nc.sync.dma_start(out=kh, in_=kvt_r[:, f, 0, :])
                nc.sync.dma_start(out=kl, in_=kvt_r[:, f, 1, :])
                return (kh.rearrange("p (h u c) -> p h u c", h=4, u=2),
                        kl.rearrange("p (h u c) -> p h u c", h=4, u=2))

            def ph3_head(f, h, khr, klr, c0, c1):
                fc = slice(f * 512, (f + 1) * 512)
                a0h = p2hi[h][:, :, 0:128]
                a0l = p2lo[h][:, :, 0:128]
                a1h = p2hi[h][:, :, 128:240]
                a1l = p2lo[h][:, :, 128:240]
                bh, bl = khr[:, h, :, :], klr[:, h, :, :]
                nc.tensor.matmul(c0, a0h, bh, start=True, stop=False,
                                 perf_mode=DR)
                nc.tensor.matmul(c0, a0l, bh, start=False, stop=False,
                                 perf_mode=DR)
                nc.tensor.matmul(c0, a0h, bl, start=False, stop=True,
                                 perf_mode=DR)
                nc.tensor.matmul(c1, a1h, bh, start=True, stop=False,
                                 perf_mode=DR)
                nc.tensor.matmul(c1, a1l, bh, start=False, stop=False,
                                 perf_mode=DR)
                nc.tensor.matmul(c1, a1h, bl, start=False, stop=True,
                                 perf_mode=DR)
                o = (h % 2) * 64
                bt = b01 if h < 2 else b23
                cd = cdA if h < 2 else cdB
                nc.scalar.copy(t4hi[:, h, fc], c0)
                nc.vector.tensor_tensor(out=t4lo[:, h, fc], in0=c0,
                                        in1=t4hi[:, h, fc], op=SUB)
                cp(h + f, tc1[h][:, fc], c1)
                nc.gpsimd.tensor_copy(bt[o:o + 64, fc], tc1[h][0:64, fc])
                nc.gpsimd.tensor_copy(cd[o:o + 48, fc], tc1[h][64:112, fc])

            # ---- phase 2: step-interleaved, with ph3(f0) fused in ----
            khr0, klr0 = kvt_load(0, gate=False)
            khr1, klr1 = kvt_load(1, gate=True)
            kvtbl = {2: kvt_load(2, gate=True)}
            ops = outer.enter_context(tc.tile_pool(name="ops", bufs=3, space="PSUM"))
            with tc.tile_pool(name="phw", bufs=4, space="PSUM") as phw, \
                 tc.tile_pool(name="phb", bufs=1, space="PSUM") as phb:
                scp, st0, st1, tiny, var, rall = {}, {}, {}, {}, {}, {}
                rv, pr0, pr1, sq0, sq1 = {}, {}, {}, {}, {}
                for h in range(H):
                    scp[h] = phw.tile([128, 2, 246], F32, tag="w", name=f"scp{h}")
                    tiny[h] = scp[h][:, 0, 242:246]
                    nc.tensor.matmul(scp[h][:, 0, 0:241], gsb[h][:, 0, 0:128],
                                     wkt_sb[:, 0, :], start=True, stop=False)
                    nc.tensor.matmul(scp[h][:, 0, 0:241], gsb[h][0:112, 1, 0:128],
                                     wkt_sb[0:112, 1, :], start=False, stop=True)
                    nc.tensor.matmul(scp[h][0:112, 1, 0:241], gsb[h][:, 0, 128:240],
                                     wkt_sb[:, 0, :], start=True, stop=False)
                    nc.tensor.matmul(scp[h][0:112, 1, 0:241],
                                     gsb[h][0:112, 1, 128:240],
                                     wkt_sb[0:112, 1, :], start=False, stop=True)
                for h in range(H):
                    st0[h] = sm.tile([128, 2], F32, tag=f"st0{h}", name=f"st0{h}")
                    st1[h] = sm.tile([112, 2], F32, tag=f"st1{h}", name=f"st1{h}")
                    nc.vector.tensor_copy(st0[h][:, 0:1], scp[h][:, 0, 240:241])
                    nc.vector.tensor_copy(st1[h][:, 0:1], scp[h][0:112, 1, 240:241])
                    sq0[h] = sm.tile([128, 240], BF16, tag="sqs", bufs=2,
                                     name=f"sq0{h}")
                    sq1[h] = sm.tile([112, 240], BF16, tag="sqs2", bufs=2,
                                     name=f"sq1{h}")
                    nc.scalar.activation(out=sq0[h], in_=scp[h][:, 0, 0:240],
                                         func=mybir.ActivationFunctionType.Square,
                                         accum_out=st0[h][:, 1:2])
                    nc.scalar.activation(out=sq1[h], in_=scp[h][0:112, 1, 0:240],
                                         func=mybir.ActivationFunctionType.Square,
                                         accum_out=st1[h][:, 1:2])
                for h in range(H):
                    nc.tensor.matmul(tiny[h][0:4, 2:4], indb_sb[:, 0:4], st0[h],
                                     start=True, stop=False)
                    nc.tensor.matmul(tiny[h][0:4, 2:4], indb_sb[0:112, 4:8], st1[h],
                                     start=False, stop=True)
                for h in range(H):
                    # indb is pre-scaled by 1/(nblk*240): tiny holds (mean, meansq)
                    var[h] = sm.tile([4, 1], F32, tag=f"var{h}", name=f"var{h}")
                    nc.scalar.activation(out=var[h], in_=tiny[h][0:4, 2:3],
                                         func=mybir.ActivationFunctionType.Square)
                    nc.vector.tensor_tensor(out=var[h], in0=tiny[h][0:4, 3:4],
                                            in1=var[h], op=SUB)
                for h in range(H):
                    rall[h] = sm.tile([4, 1], F32, tag=f"rall{h}", name=f"rall{h}")
                    nc.scalar.activation(out=rall[h], in_=var[h], func=ASqrt,
                                         bias=eps_t)
                    nc.vector.reciprocal(out=rall[h], in_=rall[h])
                for h in range(H):
                    nc.tensor.matmul(tiny[h][:, 0:1], indc_sb[:, 0:128], rall[h],
                                     start=True, stop=True)
                    nc.tensor.matmul(tiny[h][0:112, 1:2], indc_sb[:, 128:240],
                                     rall[h], start=True, stop=True)
                    rv[h] = sm.tile([128, 2], F32, tag=f"rv{h}", name=f"rv{h}")
                    nc.vector.tensor_copy(rv[h][:, 0:1], tiny[h][:, 0:1])
                    nc.vector.tensor_copy(rv[h][0:112, 1:2], tiny[h][0:112, 1:2])
                for h in range(H):
                    # inorm guarantees ~unit-variance scores: shift-free softmax
                    pr0[h] = sm.tile([128, 240], BF16, tag=f"pr0{h}", name=f"pr0{h}")
                    pr1[h] = sm.tile([112, 240], BF16, tag=f"pr1{h}", name=f"pr1{h}")
                    nc.scalar.activation(out=pr0[h], in_=scp[h][:, 0, 0:240],
                                         func=AExp, scale=rv[h][:, 0:1],
                                         accum_out=esr[h][:, 0:1])
                    nc.scalar.activation(out=pr1[h], in_=scp[h][0:112, 1, 0:240],
                                         func=AExp, scale=rv[h][0:112, 1:2],
                                         accum_out=esr[h][0:112, 1:2])
                    nc.vector.reciprocal(out=esr[h][:, 0:1], in_=esr[h][:, 0:1])
                    nc.vector.reciprocal(out=esr[h][0:112, 1:2],
                                         in_=esr[h][0:112, 1:2])
                for h in range(H):
                    # fold softmax 1/esum into P before the transpose
                    nc.gpsimd.tensor_scalar(out=pr0[h], in0=pr0[h],
                                            scalar1=esr[h][:, 0:1], scalar2=None,
                                            op0=MULT)
                    nc.gpsimd.tensor_scalar(out=pr1[h], in0=pr1[h],
                                            scalar1=esr[h][0:112, 1:2],
                                            scalar2=None, op0=MULT)
                pt = {}
                for h in range(H):
                    ptp = phb.tile([128, 2, 240], BF16, tag="b", name=f"ptp{h}")
                    nc.tensor.transpose(ptp[:, 0, 0:128], pr0[h][:, 0:128], idt)
                    nc.tensor.transpose(ptp[0:112, 1, 0:128], pr0[h][:, 128:240],
                                        idt)
                    nc.tensor.transpose(ptp[:, 0, 128:240], pr1[h][:, 0:128],
                                        idt[0:112, 0:112])
                    nc.tensor.transpose(ptp[0:112, 1, 128:240],
                                        pr1[h][:, 128:240], idt[0:112, 0:112])
                    pt[h] = sm.tile([128, 2, 240], BF16, tag=f"pt{h}",
                                    name=f"pt{h}")
                    cp(0, pt[h][:, 0, :], ptp[:, 0, :])
                    cp(0, pt[h][0:112, 1, :], ptp[0:112, 1, :])
                    p2p = phw.tile([128, 2, 246], F32, tag="w", name=f"p2p{h}")
                    nc.tensor.matmul(p2p[:, 0, 0:240], wvs_sb[:, 0, 0:128],
                                     pt[h][:, 0, :], start=True, stop=False)
                    nc.tensor.matmul(p2p[:, 0, 0:240], wvs_sb[0:112, 1, 0:128],
                                     pt[h][0:112, 1, :], start=False, stop=True)
                    nc.tensor.matmul(p2p[0:112, 1, 0:240], wvs_sb[:, 0, 128:240],
                                     pt[h][:, 0, :], start=True, stop=False)
                    nc.tensor.matmul(p2p[0:112, 1, 0:240], wvs_sb[0:112, 1, 128:240],
                                     pt[h][0:112, 1, :], start=False, stop=True)
                    # e4m3 hi/lo evac of G^T (scaled by GS via wvs)
                    nc.scalar.copy(p2hi[h][:, 0, :], p2p[:, 0, 0:240])
                    nc.vector.tensor_tensor(out=p2lo[h][:, 0, :],
                                            in0=p2p[:, 0, 0:240],
                                            in1=p2hi[h][:, 0, :], op=SUB)
                    nc.scalar.copy(p2hi[h][0:112, 1, :], p2p[0:112, 1, 0:240])
                    nc.vector.tensor_tensor(out=p2lo[h][0:112, 1, :],
                                            in0=p2p[0:112, 1, 0:240],
                                            in1=p2hi[h][0:112, 1, :], op=SUB)
                    # ph3(f0/f1) for earlier heads fills PE while this
                    # head's G evac completes
                    if h > 0:
                        c0 = ops.tile([128, 512], F32, tag="o", name=f"fc0{h}")
                        c1 = ops.tile([112, 512], F32, tag="o", name=f"fc1{h}")
                        ph3_head(0, h - 1, khr0, klr0, c0, c1)
                    if h > 1:
                        c0 = ops.tile([128, 512], F32, tag="o", name=f"gc0{h}")
                        c1 = ops.tile([112, 512], F32, tag="o", name=f"gc1{h}")
                        ph3_head(1, h - 2, khr1, klr1, c0, c1)
                for fz, hz in ((0, 3), (1, 2), (1, 3)):
                    c0 = ops.tile([128, 512], F32, tag="o", name=f"fz{fz}{hz}")
                    c1 = ops.tile([112, 512], F32, tag="o", name=f"fz{fz}{hz}b")
                    ph3_head(fz, hz, khr1 if fz else khr0, klr1 if fz else klr0,
                             c0, c1)

            # ---- phases 3+4 fully fused per f-block ----
            with tc.tile_pool(name="cps", bufs=3, space="PSUM") as cps, \
                 tc.tile_pool(name="cps1", bufs=2, space="PSUM") as cps1:
                def ph3_block(f):
                    khr, klr = kvtbl.pop(f) if f in kvtbl else kvt_load(f, False)
                    for h in range(H):
                        c0 = cps.tile([128, 512], F32, tag="c0")
                        c1 = cps1.tile([112, 512], F32, tag="c1")
                        ph3_head(f, h, khr, klr, c0, c1)

                def ph4_block(f, interleave=False):
                    ost = ostp.tile([128, 4, 960], BF16, tag="ost")
                    def bp_block(q):
                        j = 4 * f + q
                        ncol = slice(j * 128, (j + 1) * 128)
                        bp = ops.tile([128, 512], F32, tag="o", name=f"bp{j}")
                        for p in range(2):
                            hh = slice(2 * p, 2 * p + 2)
                            nc.tensor.matmul(bp, t4hi[:, hh, ncol],
                                             w4r[:, 0, p, :, :],
                                             start=(p == 0), stop=False,
                                             perf_mode=DR)
                            nc.tensor.matmul(bp, t4lo[:, hh, ncol],
                                             w4r[:, 0, p, :, :],
                                             start=False, stop=False,
                                             perf_mode=DR)
                            nc.tensor.matmul(bp, t4hi[:, hh, ncol],
                                             w4r[:, 1, p, :, :],
                                             start=False, stop=(p == 1),
                                             perf_mode=DR)
                        cpmul(q, ost[:, q, 448:960], bp, 1.0 / (GS * GW))
                    def ap_block(q, dma_pair):
                        j = 4 * f + q
                        ncol = slice(j * 128, (j + 1) * 128)
                        ap = ops.tile([128, 512], F32, tag="o", name=f"ap{j}")
                        nc.tensor.matmul(ap[:, 192:448], b01[:, ncol],
                                         w3_sb[:, 0, :], start=True, stop=False)
                        nc.tensor.matmul(ap[:, 192:448], b23[:, ncol],
                                         w3_sb[:, 1, :], start=False, stop=True)
                        nc.tensor.matmul(ap[:, 0:192], cdA[:, ncol], wcdA_sb,
                                         start=True, stop=False)
                        nc.tensor.matmul(ap[:, 0:192], cdB[:, ncol], wcdB_sb,
                                         start=False, stop=True)
                        cpmul(q + 1, ost[:, q, 0:448], ap[:, 0:448], 1.0 / GS)
                        if not dma_pair:
                            nc.sync.dma_start(out=out_r[:, j:j + 1, :],
                                              in_=ost[:, q:q + 1, :])
                        elif q % 2 == 1:
                            nc.sync.dma_start(
                                out=out_r[:, j - 1:j + 1, :],
                                in_=ost[:, q - 1:q + 1, :])
                    if interleave:
                        for q in range(4):
                            bp_block(q)
                            ap_block(q, dma_pair=False)
                    else:
                        for q in range(4):
                            bp_block(q)
                        for q in range(4):
                            ap_block(q, dma_pair=(f != NF - 1))

                # software-pipelined: ph4 consumes f-2 while ph3 produces
                # f (f0/f1 already ran fused into phase 2)
                for step in range(2, NF + 2):
                    if step < NF:
                        ph3_block(step)
                    ph4_block(step - 2, interleave=(step == NF + 1))
    nc.finalize()
    return nc


def _host_pack(inputs, b, side, qdt):
    if side == 0:
        embs = [inputs['emb1'], inputs['emb2'], inputs['emb3'], inputs['emb4']]
        wq = [inputs[f'wq{i+1}'] for i in range(4)]
        kvsrc = inputs['emb_alld']
    else:
        embs = [inputs['embd1'], inputs['embd2'], inputs['embd3'], inputs['embd4']]
        wq = [inputs[f'wqd{i+1}'] for i in range(4)]
        kvsrc = inputs['emb_all']
    scale = np.float32(1.0 / np.sqrt(np.float32(KV)))
    eq = np.empty((N, 960), np.float32)
    kvf = np.empty((N, 960), np.float32)
    for h in range(H):
        for i in range(4):
            cq = CQ[i]
            blkq = np.asarray(embs[i][b][:, h * cq:(h + 1) * cq], np.float32)
            # fold wq (and 1/sqrt(KV)) into the q-side pack: Q = emb @ wq^T
            eq[:, h * 240 + QOFF[i]: h * 240 + QOFF[i] + cq] = \
                (blkq @ np.asarray(wq[i][h], np.float32).T) * scale
            kvf[:, h * 240 + KOFF[i]: h * 240 + KOFF[i] + cq] = \
                kvsrc[b][:, RAW[i] + h * cq: RAW[i] + (h + 1) * cq]

    # kvT e4m3 hi/lo pack: [128, f hl h u c], u1 partitions 112:128 zero
    kvt = np.zeros((128, NF, 2, H, 2, 512), np.float32)
    for h in range(H):
        kT = kvf[:, h * 240:(h + 1) * 240].T  # (240, N)
        for u, (r0, r1) in enumerate(((0, 128), (128, 240))):
            blk = kT[r0:r1].reshape(r1 - r0, NF, 512)
            hi = blk.astype(E4NP).astype(np.float32)
            lo = (blk - hi).astype(E4NP).astype(np.float32)
            kvt[0:r1 - r0, :, 0, h, u, :] = hi.transpose(0, 1, 2)
            kvt[0:r1 - r0, :, 1, h, u, :] = lo
    kvthl = np.ascontiguousarray(kvt.reshape(128, -1)).astype(E4NP)

    if qdt is E3:
        # per-head power-of-2 scaling into the e3m4 sweet spot (absmax ~12);
        # inorm makes S invariant to any per-head constant factor
        for h in range(H):
            cs = slice(h * 240, (h + 1) * 240)
            eq[:, cs] *= 2.0 ** np.floor(np.log2(12.0 / np.abs(eq[:, cs]).max()))
            kvf[:, cs] *= 2.0 ** np.floor(np.log2(12.0 / np.abs(kvf[:, cs]).max()))
        qnp = E3NP
    else:
        qnp = BF
    eqp = np.ascontiguousarray(
        eq.reshape(NT, 128, 960).transpose(1, 0, 2)).reshape(128, NT * 960)
    kvp = np.ascontiguousarray(
        kvf.reshape(NT, 128, 960).transpose(1, 0, 2)).reshape(128, NT * 960)
    return eqp.astype(qnp), kvp.astype(qnp), kvthl


def _host_weights(inputs, side):
    if side == 0:
        wk, wv = inputs['wkd'], inputs['wvd']
        wout = [inputs[f'wout{i+1}'] for i in range(4)]
    else:
        wk, wv = inputs['wk'], inputs['wv']
        wout = [inputs[f'woutd{i+1}'] for i in range(4)]
    wkT = np.asarray(wk).T.astype(np.float32)
    wkt = np.zeros((128, 2, 241), np.float32)
    wkt[:, 0, 0:240] = wkT[0:128, :]
    wkt[0:112, 1, 0:240] = wkT[128:240, :]
    wkt[:, 0, 240] = wkT[0:128, :].sum(axis=1)
    wkt[0:112, 1, 240] = wkT[128:240, :].sum(axis=1)
    wvf = np.asarray(wv).astype(np.float32) * np.float32(GS)
    wvs = np.zeros((128, 2, 240), np.float32)
    wvs[:, 0, :] = wvf[0:128, :]
    wvs[0:112, 1, :] = wvf[128:240, :]
    w3t = np.asarray(wout[2]).T.astype(np.float32)
    w3pk = np.stack([w3t[0:128, :], w3t[128:256, :]], axis=1)
    w2pk = np.asarray(wout[1]).T.astype(np.float32)
    w1t = np.asarray(wout[0]).T.astype(np.float32)
    wcd = np.zeros((2, 128, 192), np.float32)
    for h in range(H):
        t, o = divmod(h, 2)
        wcd[t, o * 64 + 0:o * 64 + 32, 64:192] = w2pk[h * 32:(h + 1) * 32, :]
        wcd[t, o * 64 + 32:o * 64 + 48, 0:64] = w1t[h * 16:(h + 1) * 16, :]
    wallh = np.zeros((128, W_END), np.float32)
    wallh[:, 0:128] = np.eye(128, dtype=np.float32)
    wallh[:, W_WKT:W_WVS] = wkt.reshape(128, 482)
    wallh[:, W_WVS:W_W3] = wvs.reshape(128, 480)
    wallh[:, W_W3:W_CDA] = w3pk.reshape(128, 512)
    wallh[:, W_CDA:W_CDB] = wcd[0]
    wallh[:, W_CDB:W_ZZ] = wcd[1]

    # w4 * GW as e4m3 hi/lo: [128, hl pair kt c]
    w4t = np.asarray(wout[3]).T.astype(np.float32) * np.float32(GW)
    w4pk = np.stack([w4t[h * 128:(h + 1) * 128, :] for h in range(H)], axis=1)
    w4a = np.zeros((128, 2, 2, 2, 512), np.float32)
    for p in range(2):
        for k in range(2):
            blk = w4pk[:, 2 * p + k, :]
            hi = blk.astype(E4NP).astype(np.float32)
            w4a[:, 0, p, k, :] = hi
            w4a[:, 1, p, k, :] = (blk - hi).astype(E4NP).astype(np.float32)
    w4hl = np.ascontiguousarray(w4a.reshape(128, 4096)).astype(E4NP)
    return dict(wall=wallh.astype(BF), w4hl=w4hl)


def _host_consts():
    # indb pre-scaled by 1/(nblk*240) so the indicator matmul yields means
    indb = np.zeros((128, 8), np.float32)
    indb[:, 0] = 1.0 / (128 * 240)
    indb[0:64, 5] = 1.0 / (64 * 240)
    indb[64:96, 6] = 1.0 / (32 * 240)
    indb[96:112, 7] = 1.0 / (16 * 240)
    indc = np.zeros((4, 240), np.float32)
    indc[0, 0:128] = 1.0
    indc[1, 128:192] = 1.0
    indc[2, 192:224] = 1.0
    indc[3, 224:240] = 1.0
    return dict(indb=indb, indc=indc)


def _in_map(inputs, b, side, wside, consts):
    eqp, kvp, kvthl = _host_pack(inputs, b, side, QDT)
    return dict(eqp=eqp, kvp=kvp, kvthl=kvthl, **wside[side], **consts)


def _unpack_out(raw):
    o = np.asarray(raw).reshape(128, NT, 960).transpose(1, 0, 2)
    return np.ascontiguousarray(o).reshape(N, 960).astype(np.float32)


def kernel(**inputs):
    inputs = {k: np.asarray(v, dtype=np.float32) for k, v in inputs.items()}
    key = ("nc", str(QDT))
    if key not in _CACHE:
        _CACHE[key] = _build_bass(QDT)
        _CACHE["nc"] = _CACHE[key]
    nc = _CACHE[key]
    consts = _host_consts()
    wside = [_host_weights(inputs, 0), _host_weights(inputs, 1)]
    in_maps = [_in_map(inputs, core // 2, core % 2, wside, consts)
               for core in range(8)]
    res = run_bass_kernel_spmd(nc, in_maps, list(range(8)))
    out = np.empty((B, N, 2 * KV), np.float32)
    for core in range(8):
        b, side = core // 2, core % 2
        out[b, :, side * 960:(side + 1) * 960] = _unpack_out(res.results[core]["outp"])
    return out
